# revision 1
# baseline (speedup 1.0000x reference)
"""Trainium2 Bass kernel for nn_MultiHeadDiffAttention (B=2,T=2048,C=1024,H=16).

Sharding: 8 cores = data-parallel over B(2) x tensor-parallel over 4 head-groups
(4 heads each). Each core computes q/k/v projections for its heads, causal
differential attention, per-head GroupNorm, and a partial output projection
(its 512 columns of y2 against Wc). Host sums the 4 partials per batch.

v2 layout strategy per core:
  - x passed transposed+bf16 from host: xT [C=1024, T=2048].
  - qT/kT per head [64, T] via matmul(lhsT=W^T chunk, rhs=xT chunk).
  - v in [t, d] layout via matmul(lhsT=xT chunk, rhs=WvT); interleaved with
    head-0 attention to keep PE busy while ACT runs exp.
  - S tiles [q=128, k<=512] on PE (bf16), causal-trimmed; diagonal 128-block
    masked via an additive -30000 mask matmul before exp.
  - ACT exp with accum_out gives softmax denominators for free.
  - combine: att' = e1 + s*e2 with s = -lam*D1/D2 (per-partition scalar) via
    tensor_scalar + tensor_tensor on DVE (both 2x-capable); the remaining
    1/D1 normalization is folded into the PE transpose, which becomes a
    regular matmul against diag(r1) instead of an identity transpose:
    attT = att'^T @ diag(r1), same PE cost as a plain transpose.
  - attT (f32 psum) -> ab (bf16 sbuf) casts split across DVE/ACT.
  - z = att @ v accumulates on PE into zT [d, q] per q-block.
  - GroupNorm stats via ACT accum on eviction; rsqrt via DVE Newton;
    affine applied on DVE (tensor_scalar mult+add per-partition scalars).
  - partial out^T [o=1024, t] = Wc_slice^T.T @ yT accumulated over 4 heads,
    evicted on DVE, DMA'd out per chunk.
Host gathers: out[b] = sum over head-group cores of outT.T.
"""

import sys

for _p in ("/opt/trn_rl_repo", "/root/.axon_site/_ro/trn_rl_repo"):
    if _p not in sys.path:
        sys.path.insert(0, _p)

import math
import numpy as np
import ml_dtypes

import concourse.bass as bass
import concourse.bacc as bacc
import concourse.tile as tile
import concourse.mybir as mybir
from concourse import bass_utils

F32 = mybir.dt.float32
BF16 = mybir.dt.bfloat16
AF = mybir.ActivationFunctionType
ALU = mybir.AluOpType

B, T, C = 2, 2048, 1024
H = 16
HS = C // H           # 64
D = 2 * HS            # 128 v-channels per head
NH = 4                # heads per core
N_CORES = 8
NT = T // 128         # 16 q-tiles
LAMBDA_INIT = 0.8 - 0.6 * math.exp(-0.3 * (12 - 1))
EPS = 1e-5
SCALE = 1.0 / math.sqrt(HS)
NEG = -30000.0

_cache = {}


def _build(T=T, trace_sim=False, stage=5, nh=NH):
    # stage: 1=proj only, 2=+S/exp/combine, 3=+transpose+z, 4=+groupnorm,
    #        5=full (out-proj). nh: number of heads to process (debug).
    NT = T // 128
    nc = bacc.Bacc("TRN2", target_bir_lowering=False, debug=False,
                   num_devices=N_CORES)

    def din(name, shape, dt=BF16):
        return nc.dram_tensor(name, shape, dt, kind="ExternalInput").ap()

    xT_d = din("xT", [C, T])
    # q/k/v weights arrive in host-merged tile layout: [2 tiles x 128
    # partitions, 4 chunks x cols] so each needs only 2 DMAs
    wq1_d = din("wq1T", [256, 4 * NH * HS])
    wq2_d = din("wq2T", [256, 4 * NH * HS])
    wk1_d = din("wk1T", [256, 4 * NH * HS])
    wk2_d = din("wk2T", [256, 4 * NH * HS])
    wv_d = din("wvT", [256, 4 * NH * D])
    wc_d = din("wcT", [NH * D, C])
    mask_d = din("mask128", [128, 128])
    ident_d = din("ident", [128, 128])
    gg_d = din("gg", [128, 128], F32)
    gw2_d = din("gw2", [128, 1], F32)
    gb2_d = din("gb2", [128, 1], F32)
    lamn_d = din("lamn", [128, NH], F32)
    icon_d = din("icon", [128, 2], mybir.dt.uint32)
    outT_d = nc.dram_tensor("outT", [C, T], F32, kind="ExternalOutput").ap()
    dbg_d = (nc.dram_tensor("dbg", [128, T], F32, kind="ExternalOutput").ap()
             if stage != 5 else None)

    with tile.TileContext(nc, trace_sim=trace_sim) as tc:
        with tc.tile_pool(name="persist", bufs=1) as pp, \
             tc.tile_pool(name="ps_s", bufs=3, space="PSUM") as ps_s, \
             tc.tile_pool(name="ps_t", bufs=2, space="PSUM") as ps_t:

            # ---- PE warmup: ramp the tensor-engine P-state during the
            # input DMA window with dummy back-to-back matmuls ----
            wa_t = pp.tile([128, 256], BF16, tag="wa")
            nc.gpsimd.memset(wa_t[:], 0.0)
            for _ in range(64):
                psw = ps_s.tile([128, 1024], F32, tag="s")
                nc.tensor.matmul(psw[:, 0:256], wa_t[:, 0:128], wa_t[:],
                                 start=True, stop=True)

            # ---- persistent small tiles ----
            mask_t = pp.tile([128, 128], BF16, tag="mask")
            nc.sync.dma_start(mask_t[:], mask_d)
            ident_t = pp.tile([128, 128], BF16, tag="ident")
            nc.sync.dma_start(ident_t[:], ident_d)
            gg_t = pp.tile([128, 128], F32, tag="gg")
            nc.gpsimd.dma_start(gg_t[:], gg_d)
            gw2_t = pp.tile([128, 1], F32, tag="gw2")
            nc.gpsimd.dma_start(gw2_t[:], gw2_d)
            gb2_t = pp.tile([128, 1], F32, tag="gb2")
            nc.gpsimd.dma_start(gb2_t[:], gb2_d)
            lamn_t = pp.tile([128, NH], F32, tag="lamn")
            nc.sync.dma_start(lamn_t[:], lamn_d)
            c15_t = pp.tile([128, 1], F32, tag="c15")
            nc.vector.memset(c15_t[:], 1.5)
            icon_t = pp.tile([128, 2], mybir.dt.uint32, tag="icon")
            nc.gpsimd.dma_start(icon_t[:], icon_d)

            # ---- persistent activation tensors ----
            # qT/kT: [NH*HS=256, T] as 2 partition-tiles of 128
            q1t = [pp.tile([128, T], BF16, tag=f"q1t{i}", name=f"q1t{i}") for i in range(2)]
            q2t = [pp.tile([128, T], BF16, tag=f"q2t{i}", name=f"q2t{i}") for i in range(2)]
            k1t = [pp.tile([128, T], BF16, tag=f"k1t{i}", name=f"k1t{i}") for i in range(2)]
            k2t = [pp.tile([128, T], BF16, tag=f"k2t{i}", name=f"k2t{i}") for i in range(2)]
            # v: [T, NH*D=512] as 16 t-chunk tiles
            vt = [pp.tile([128, NH * D], BF16, tag=f"vt{i}", name=f"vt{i}") for i in range(NT)]
            # yT per head [D=128, T] bf16 (post-groupnorm)
            yt = [pp.tile([128, T], BF16, tag=f"yt{j}", name=f"yt{j}") for j in range(NH)]
            # wcT: [512, C] as 4 f-chunk tiles (one per head)
            wct = [pp.tile([128, C], BF16, tag=f"wct{j}", name=f"wct{j}") for j in range(NH)]

            # attention working pool opened BEFORE the loads pool so the
            # loads pool can be released first (pools close LIFO)
            wp_cm = tc.tile_pool(name="aw", bufs=2)
            wp = wp_cm.__enter__()

            # ================= q/k projection loads =================
            lp_cm = tc.tile_pool(name="loads", bufs=1)
            lp = lp_cm.__enter__()
            xt = [lp.tile([128, T], BF16, tag=f"xt{i}", name=f"xt{i}") for i in range(8)]
            # merged weight tiles: index [half][:, (cc%4)*cols + ...]
            wqm = {}
            for nm in ("q1", "q2", "k1", "k2"):
                wqm[nm] = [lp.tile([128, 4 * NH * HS], BF16,
                                   tag=f"w{nm}{h}", name=f"w{nm}{h}")
                           for h in range(2)]
            wvm = [lp.tile([128, 4 * NH * D], BF16, tag=f"wvm{h}",
                           name=f"wvm{h}") for h in range(2)]

            def wq_sl(nm, cc, oc):
                return wqm[nm][cc // 4][:, (cc % 4) * 256 + oc * 128:
                                        (cc % 4) * 256 + oc * 128 + 128]

            def wv_sl(cc):
                return wvm[cc // 4][:, (cc % 4) * 512:(cc % 4) * 512 + 512]

            # x as 8 x 512KB DMAs on the SP queue; weights split 4-ways per
            # tensor across the ACT and Pool queues for DMA-engine
            # parallelism without serializing any single issue queue
            for i in range(8):
                nc.sync.dma_start(xt[i][:], xT_d[i * 128:(i + 1) * 128, :])
            for nm, d_ap, eng in (("q1", wq1_d, nc.scalar),
                                  ("k1", wk1_d, nc.scalar),
                                  ("q2", wq2_d, nc.gpsimd),
                                  ("k2", wk2_d, nc.gpsimd)):
                for h in range(2):
                    for cf in range(2):
                        eng.dma_start(
                            wqm[nm][h][:, cf * 512:(cf + 1) * 512],
                            d_ap[h * 128:(h + 1) * 128,
                                 cf * 512:(cf + 1) * 512])
            for h in range(2):
                for cf in range(2):
                    nc.sync.dma_start(
                        wvm[h][:, cf * 1024:(cf + 1) * 1024],
                        wv_d[h * 128:(h + 1) * 128,
                             cf * 1024:(cf + 1) * 1024])
            for j in range(NH):
                nc.gpsimd.dma_start(wct[j][:], wc_d[j * 128:(j + 1) * 128, :])

            # qT/kT projections: out [o=128, t=512] = W^T_chunk.T @ xT
            # tb is the inner loop so 4 consecutive matmuls share the same
            # stationary operand; the 4 t-blocks accumulate in 4 independent
            # psum banks (2x [128,1024] tiles). oc=0 (heads 0-1) runs up
            # front; oc=1 (heads 2-3) is deferred into the head-0/1 loops
            # via the bg queue so it overlaps their ACT-heavy exp phases.
            _qk_ps = {}

            def emit_qk(nm, dst, oc, half):
                if half == 0:
                    _qk_ps[(nm, oc)] = (ps_s.tile([128, 1024], F32, tag="s", name="qkA"),
                                        ps_s.tile([128, 1024], F32, tag="s", name="qkB"))
                psA, psB = _qk_ps[(nm, oc)]
                for cc in range(4 * half, 4 * half + 4):
                    for tb in range(T // 512):
                        ph = (psA, psB)[tb // 2]
                        mi = nc.tensor.matmul(
                            ph[:, (tb % 2) * 512:(tb % 2) * 512 + 512],
                            wq_sl(nm, cc, oc),
                            xt[cc][:, tb * 512:(tb + 1) * 512],
                            start=(cc == 0), stop=(cc == 7),
                            skip_group_check=True)
                        if tb > 0:
                            # same stationary as tb=0: skip LDWEIGHTS
                            mi.ins.ldweights = False
                if half == 1:
                    for tb in range(T // 512):
                        ph = (psA, psB)[tb // 2]
                        src = ph[:, (tb % 2) * 512:(tb % 2) * 512 + 512]
                        if nm in ("q1", "k1"):
                            nc.scalar.copy(
                                dst[oc][:, tb * 512:(tb + 1) * 512], src)
                        else:
                            nc.vector.tensor_copy(
                                dst[oc][:, tb * 512:(tb + 1) * 512], src)

            for nm, dst in (("q1", q1t), ("k1", k1t), ("q2", q2t), ("k2", k2t)):
                emit_qk(nm, dst, 0, 0)
                emit_qk(nm, dst, 0, 1)

            # v projection chunks are emitted lazily (interleaved with head 0)
            v_done = [False] * NT

            def emit_v(tch):
                if v_done[tch]:
                    return
                v_done[tch] = True
                ps = ps_t.tile([128, NH * D], F32, tag="tz")
                for cc in range(8):
                    nc.tensor.matmul(
                        ps[:],
                        xt[cc][:, tch * 128:(tch + 1) * 128],
                        wv_sl(cc),
                        start=(cc == 0), stop=(cc == 7))
                if tch % 2 == 0:
                    nc.vector.tensor_copy(vt[tch][:], ps[:])
                else:
                    nc.scalar.copy(vt[tch][:], ps[:])

            # background PE work drained between attention phases of
            # heads 0-1: all v chunks, then the oc=1 q/k projections
            def emit_qk_full(nm, dst):
                emit_qk(nm, dst, 1, 0)
                emit_qk(nm, dst, 1, 1)

            bg = [(lambda t=t: emit_v(t)) for t in range(NT)]
            for nm, dst in (("q1", q1t), ("k1", k1t), ("q2", q2t), ("k2", k2t)):
                bg.append(lambda nm=nm, dst=dst: emit_qk_full(nm, dst))

            def bg_drain(n):
                for _ in range(min(n, len(bg))):
                    bg.pop(0)()

            # ================= attention per head =================
            if stage == 1:
                dbg_t = wp.tile([128, T], F32, tag="dbg_t")
                nc.vector.tensor_copy(dbg_t[:], q1t[0][:])
                nc.sync.dma_start(dbg_d, dbg_t[:])
            head_ctx = {}

            def phase1(j, qb):
                oc, po = divmod(j * HS, 128)
                att_rows = {}
                for qq in range(4):
                    qt = qb * 4 + qq
                    nk = qt + 1
                    nkb2 = (nk + 7) // 8   # 1024-wide S psum tiles
                    # e1 doubles as att' (combined in-place), so its ring
                    # must cover the LAG pipeline depth (2 qb x 4 qt)
                    e1 = wp.tile([128, T], BF16, tag="e1", name="e1", bufs=8)
                    e2 = wp.tile([128, T], BF16, tag="e2", name="e2", bufs=2)
                    d1c = wp.tile([128, 2], F32, tag="d1c", name="d1c")
                    d2c = wp.tile([128, 2], F32, tag="d2c", name="d2c")
                    for mi, (qsrc, ksrc, erow, dcol) in enumerate(
                            ((q1t, k1t, e1, d1c), (q2t, k2t, e2, d2c))):
                        for kb in range(nkb2):
                            w = min(1024, nk * 128 - kb * 1024)
                            ps = ps_s.tile([128, 1024], F32, tag="s",
                                           name="ps")
                            off = qt * 128 - kb * 1024  # diag block col
                            for hf in range(2):
                                wh = min(512, w - hf * 512)
                                if wh <= 0:
                                    break
                                diag_here = (kb == nkb2 - 1 and
                                             hf * 512 <= off < hf * 512 + wh)
                                mi = nc.tensor.matmul(
                                    ps[:, hf * 512:hf * 512 + wh],
                                    qsrc[oc][po:po + HS,
                                             qt * 128:(qt + 1) * 128],
                                    ksrc[oc][po:po + HS,
                                             kb * 1024 + hf * 512:
                                             kb * 1024 + hf * 512 + wh],
                                    start=True, stop=not diag_here,
                                    skip_group_check=diag_here)
                                if kb + hf > 0:
                                    # same q-slice stationary as the first
                                    # chunk of this (qt, matrix) row
                                    mi.ins.ldweights = False
                            if kb == nkb2 - 1:
                                # mask diagonal 128-block on PE
                                nc.tensor.matmul(
                                    ps[:, off:off + 128],
                                    ident_t[:], mask_t[:],
                                    start=False, stop=True,
                                    skip_group_check=True)
                            nc.scalar.activation(
                                erow[:, kb * 1024:kb * 1024 + w],
                                ps[:, :w], AF.Exp, scale=SCALE,
                                accum_out=dcol[:, kb:kb + 1])
                    # denominators -> rr = [1/D1, 1/D2]
                    rr = wp.tile([128, 2], F32, tag="rr", name="rr")
                    dd = wp.tile([128, 2], F32, tag="dd", name="dd")
                    if nkb2 == 1:
                        nc.vector.tensor_copy(dd[:, 0:1], d1c[:, 0:1])
                        nc.vector.tensor_copy(dd[:, 1:2], d2c[:, 0:1])
                    else:
                        nc.vector.tensor_reduce(
                            dd[:, 0:1], d1c[:, 0:nkb2],
                            axis=mybir.AxisListType.X, op=ALU.add)
                        nc.vector.tensor_reduce(
                            dd[:, 1:2], d2c[:, 0:nkb2],
                            axis=mybir.AxisListType.X, op=ALU.add)
                    nc.vector.reciprocal(rr[:], dd[:, 0:2])
                    # s = -lam * D1 / D2 (per-partition scalar)
                    sc = wp.tile([128, 1], F32, tag="sc", name="sc")
                    nc.vector.scalar_tensor_tensor(
                        sc[:], dd[:, 0:1], lamn_t[:, j:j + 1], rr[:, 1:2],
                        op0=ALU.mult, op1=ALU.mult)
                    # diag(r1) bf16 for the normalizing transpose
                    dg = wp.tile([128, 128], BF16, tag="dg", name="dg",
                                 bufs=8)
                    nc.vector.tensor_scalar(dg[:], ident_t[:], rr[:, 0:1],
                                            None, op0=ALU.mult)
                    # att' = e1 + s*e2, combined in-place into e1 with a
                    # single scalar_tensor_tensor pass
                    nc.vector.scalar_tensor_tensor(
                        e1[:, :nk * 128], e2[:, :nk * 128], sc[:, 0:1],
                        e1[:, :nk * 128], op0=ALU.mult, op1=ALU.add)
                    att_rows[qt] = (e1, dg)
                return att_rows

            def phase2(j, qb, att_rows):
                ytr, s1p, s2p = head_ctx[j]
                nkc = qb * 4 + 4
                ablk = []
                for kc in range(nkc):
                    pt = ps_t.tile([128, 512], F32, tag="tz", name="pt")
                    zw = max(0, (kc - qb * 4) * 128)
                    for qq in range(4):
                        qt = qb * 4 + qq
                        if qt >= kc:
                            att, dg = att_rows[qt]
                            # attT block = att'^T @ diag(r1): transpose +
                            # normalize in one PE op
                            nc.tensor.matmul(
                                pt[:, qq * 128:qq * 128 + 128],
                                att[:, kc * 128:kc * 128 + 128],
                                dg[:],
                                start=True, stop=True,
                                skip_group_check=True)
                    ab = wp.tile([128, 512], BF16, tag=f"atb{kc}",
                                 name=f"atb{kc}", bufs=1)
                    if kc % 4 == 3:
                        nc.scalar.copy(ab[:, zw:], pt[:, zw:])
                    else:
                        nc.vector.tensor_copy(ab[:, zw:], pt[:, zw:])
                    ablk.append((ab, zw))
                # yT[d, qblk] = sum_kc v_kc.T @ attT_kc   (N=512)
                py = ps_t.tile([128, 512], F32, tag="tz", name="py")
                for kc in range(nkc):
                    ab, zw = ablk[kc]
                    nc.tensor.matmul(
                        py[:, zw:],
                        vt[kc][:, j * 128:(j + 1) * 128],
                        ab[:, zw:],
                        start=(kc == 0), stop=(kc == nkc - 1),
                        skip_group_check=True)
                # copy to ytr with fused stats accumulation (ACT)
                nc.scalar.activation(
                    ytr[:, qb * 512:(qb + 1) * 512], py[:], AF.Copy,
                    accum_out=s1p[:, qb:qb + 1])
                ysq = wp.tile([128, 512], BF16, tag="ysq", name="ysq",
                              bufs=1)
                nc.scalar.activation(
                    ysq[:], py[:], AF.Square,
                    accum_out=s2p[:, qb:qb + 1])

            def gn_final(j):
                ytr, s1p, s2p = head_ctx[j]
                if stage == 3:
                    if j == 0:
                        dbg_t = wp.tile([128, T], F32, tag="dbg_t")
                        nc.vector.tensor_copy(dbg_t[:], ytr[:])
                        nc.sync.dma_start(dbg_d, dbg_t[:])
                    return
                s12 = wp.tile([128, 2], F32, tag="s12", name="s12")
                nc.vector.tensor_reduce(s12[:, 0:1], s1p[:, 0:NT // 4],
                                        axis=mybir.AxisListType.X, op=ALU.add)
                nc.vector.tensor_reduce(s12[:, 1:2], s2p[:, 0:NT // 4],
                                        axis=mybir.AxisListType.X, op=ALU.add)
                pg = ps_t.tile([128, 2], F32, tag="tz", name="pg")
                nc.tensor.matmul(pg[:], gg_t[:], s12[:], start=True, stop=True)
                # mneg = -mean; nvar = mean^2 - E[y^2] = -var
                mneg = wp.tile([128, 1], F32, tag="mneg", name="mneg")
                nc.scalar.mul(mneg[:], pg[:, 0:1], -1.0 / (T * 4))
                msq = wp.tile([128, 1], F32, tag="msq")
                nc.scalar.mul(msq[:], pg[:, 1:2], 1.0 / (T * 4))
                nvar = wp.tile([128, 1], F32, tag="nvar")
                nc.vector.scalar_tensor_tensor(
                    nvar[:], mneg[:], mneg[:, 0:1], msq[:],
                    op0=ALU.mult, op1=ALU.subtract)
                vpe = wp.tile([128, 1], F32, tag="vpe")
                nc.vector.tensor_scalar(vpe[:], nvar[:], -1.0, EPS,
                                        op0=ALU.mult, op1=ALU.add)  # var+eps
                # rsqrt(var+eps) on DVE only: quake seed + Newton iters
                rstd = wp.tile([128, 1], F32, tag="rstd")
                yi = wp.tile([128, 1], F32, tag="yi")
                nc.vector.tensor_tensor(yi.bitcast(mybir.dt.uint32)[:],
                                        vpe.bitcast(mybir.dt.uint32)[:],
                                        icon_t[:, 0:1],
                                        ALU.logical_shift_right)
                nc.vector.tensor_tensor(yi.bitcast(mybir.dt.uint32)[:],
                                        icon_t[:, 1:2],
                                        yi.bitcast(mybir.dt.uint32)[:],
                                        ALU.subtract)
                vneg = wp.tile([128, 1], F32, tag="vneg")
                nc.vector.tensor_scalar_mul(vneg[:], vpe[:], -0.5)
                ytmp = wp.tile([128, 1], F32, tag="ytmp")
                for _ in range(3):
                    nc.vector.tensor_tensor(ytmp[:], yi[:], yi[:], ALU.mult)
                    nc.vector.scalar_tensor_tensor(
                        ytmp[:], ytmp[:], vneg[:, 0:1], c15_t[:],
                        op0=ALU.mult, op1=ALU.add)  # 1.5 - 0.5 v y^2
                    nc.vector.tensor_tensor(yi[:], yi[:], ytmp[:], ALU.mult)
                nc.vector.tensor_copy(rstd[:], yi[:])
                aff_a = wp.tile([128, 1], F32, tag="aff_a")
                nc.vector.tensor_tensor(aff_a[:], rstd[:], gw2_t[:], ALU.mult)
                aff_b = wp.tile([128, 1], F32, tag="aff_b")
                nc.vector.scalar_tensor_tensor(
                    aff_b[:], mneg[:], aff_a[:, 0:1], gb2_t[:],
                    op0=ALU.mult, op1=ALU.add)  # gb2 - mean*aff_a
                # affine on DVE (2x): yt = ytr*aff_a + aff_b
                nc.vector.tensor_scalar(yt[j][:], ytr[:], aff_a[:, 0:1],
                                        aff_b[:, 0:1],
                                        op0=ALU.mult, op1=ALU.add)
                if stage == 4 and j == 0:
                    dbg_t = wp.tile([128, T], F32, tag="dbg_t")
                    nc.vector.tensor_copy(dbg_t[:], yt[0][:])
                    nc.sync.dma_start(dbg_d, dbg_t[:])

            # driver: software-pipelined, phase2 lags phase1 by LAG q-blocks;
            # gn_final lags a further 2 drains so its tiny PE matmul (which
            # waits on a long ACT->DVE chain) never blocks the next head's
            # S matmuls in the in-order PE queue
            LAG = 1
            pend = []
            gn_pend = []

            def drain_one(defer_gn=True):
                if gn_pend and gn_pend[0][1] <= 0:
                    gn_final(gn_pend.pop(0)[0])
                for i in range(len(gn_pend)):
                    gn_pend[i][1] -= 1
                u = pend.pop(0)
                phase2(*u)
                if u[1] == NT // 4 - 1:
                    if defer_gn:
                        gn_pend.append([u[0], 2])
                    else:
                        gn_final(u[0])

            # heads processed in interleaved pairs: (0,1) then (2,3). Two
            # independent heads in flight give the PE cross-head work to
            # chew whenever one head's exp/combine chain lags, at no extra
            # SBUF cost (same ring depths as single-head LAG=1).
            hseq = []
            for p0 in range(0, nh, 2):
                pair = [p0] if p0 + 1 >= nh else [p0, p0 + 1]
                for qb in range(NT // 4):
                    for j in pair:
                        hseq.append((j, qb))
            for j, qb in (hseq if stage >= 2 else []):
                if qb == 0:
                    head_ctx[j] = (
                        wp.tile([128, T], BF16, tag="ytr", name="ytr",
                                bufs=3),
                        wp.tile([128, 4], F32, tag="s1p", name="s1p"),
                        wp.tile([128, 4], F32, tag="s2p", name="s2p"))
                att_rows = phase1(j, qb)
                if j < 2:
                    # interleave v-projection / oc=1 q,k-projection
                    # chunks so the PE stays busy while ACT runs exps
                    bg_drain(3)
                if stage == 2:
                    if j == 0 and qb == NT // 4 - 1:
                        dbg_t = wp.tile([128, T], F32, tag="dbg_t")
                        nc.vector.tensor_copy(dbg_t[:],
                                              att_rows[NT - 1][0][:])
                        nc.sync.dma_start(dbg_d, dbg_t[:])
                    continue
                pend.append((j, qb, att_rows))
                if len(pend) > LAG:
                    drain_one()
            while pend:
                drain_one()
            bg_drain(len(bg))
            while len(gn_pend) > 1:
                gn_final(gn_pend.pop(0)[0])
            lp_cm.__exit__(None, None, None)
            # drain-phase pool reuses the space just freed by "loads"
            dp_cm = tc.tile_pool(name="drain", bufs=1)
            dp = dp_cm.__enter__()

            # ================= output projection =================
            # tb inner so 4 consecutive matmuls share stationary wct slice;
            # 4 psum banks (2x [128,1024] s-tiles) per ocb, ring-3 overlaps
            # the next ocb's matmuls with this one's evictions. The last
            # head's gn_final is emitted between ocb0-1's j=0..2 partial
            # accumulations and their j=3 matmuls, so its ACT->DVE chain
            # hides behind real PE work instead of stalling the queue.
            def op_mms(ocb, halves, js, stop_j):
                for j in js:
                    for tb in range(T // 512):
                        ph, off = halves[tb]
                        mi = nc.tensor.matmul(
                            ph[:, off:off + 512],
                            wct[j][:, ocb * 128:(ocb + 1) * 128],
                            yt[j][:, tb * 512:(tb + 1) * 512],
                            start=(j == 0), stop=(j == stop_j),
                            skip_group_check=True)
                        if tb > 0:
                            mi.ins.ldweights = False

            def op_fin(ocb, halves):
                for tb in range(T // 512):
                    ph, off = halves[tb]
                    # deep ob ring in the drain pool (space freed by the
                    # loads pool) so matmuls never wait on evictions/DMA
                    ob = dp.tile([128, 512], F32, tag="ob", bufs=8,
                                 name="ob")
                    if tb % 2 == 0:
                        nc.vector.tensor_copy(ob[:], ph[:, off:off + 512])
                    else:
                        nc.scalar.copy(ob[:], ph[:, off:off + 512])
                    eng = nc.sync if tb % 2 == 0 else nc.gpsimd
                    eng.dma_start(
                        outT_d[ocb * 128:(ocb + 1) * 128,
                               tb * 512:(tb + 1) * 512], ob[:])

            def s_halves():
                psA = ps_s.tile([128, 1024], F32, tag="s", name="psA")
                psB = ps_s.tile([128, 1024], F32, tag="s", name="psB")
                return [(psA, 0), (psA, 512), (psB, 0), (psB, 512)]

            if stage == 5:
                # prefix: j=0..2 accumulation for ocb0 (all 4 t-blocks) and
                # ocb1 (first 2 t-blocks) — 18 matmuls of PE work that hide
                # the last head's gn ACT->DVE chain. The tz ring stays free
                # for gn's own pg matmul.
                hv0 = s_halves()
                pC = ps_s.tile([128, 1024], F32, tag="s", name="pC")
                op_mms(0, hv0, range(NH - 1), -1)
                for j in range(NH - 1):
                    for tb in (0, 1):
                        nc.tensor.matmul(
                            pC[:, tb * 512:tb * 512 + 512],
                            wct[j][:, 128:256],
                            yt[j][:, tb * 512:(tb + 1) * 512],
                            start=(j == 0), stop=False,
                            skip_group_check=True)
                while gn_pend:
                    gn_final(gn_pend.pop(0)[0])
                # finish ocb0, then ocb1 (tb0/1 close in pC, tb2/3 in tz)
                op_mms(0, hv0, [NH - 1], NH - 1)
                op_fin(0, hv0)
                for tb in (0, 1):
                    nc.tensor.matmul(
                        pC[:, tb * 512:tb * 512 + 512],
                        wct[NH - 1][:, 128:256],
                        yt[NH - 1][:, tb * 512:(tb + 1) * 512],
                        start=False, stop=True, skip_group_check=True)
                tA = ps_t.tile([128, 512], F32, tag="tz", name="tA")
                tB = ps_t.tile([128, 512], F32, tag="tz", name="tB")
                hv1 = [(pC, 0), (pC, 512), (tA, 0), (tB, 0)]
                for j in range(NH):
                    for tb in (2, 3):
                        ph, off = hv1[tb]
                        nc.tensor.matmul(
                            ph[:, off:off + 512],
                            wct[j][:, 128:256],
                            yt[j][:, tb * 512:(tb + 1) * 512],
                            start=(j == 0), stop=(j == NH - 1),
                            skip_group_check=True)
                op_fin(1, hv1)
                for ocb in range(2, 8):
                    halves = s_halves()
                    op_mms(ocb, halves, range(NH), NH - 1)
                    op_fin(ocb, halves)
            else:
                while gn_pend:
                    gn_final(gn_pend.pop(0)[0])
            dp_cm.__exit__(None, None, None)
            wp_cm.__exit__(None, None, None)

    nc.compile()
    return nc


def _prep_inputs(inputs):
    bf = ml_dtypes.bfloat16
    x = np.asarray(inputs["x"], np.float32)
    Wq1 = np.asarray(inputs["Wq1"], np.float32)
    Wq2 = np.asarray(inputs["Wq2"], np.float32)
    Wk1 = np.asarray(inputs["Wk1"], np.float32)
    Wk2 = np.asarray(inputs["Wk2"], np.float32)
    Wv = np.asarray(inputs["Wv"], np.float32)
    Wc = np.asarray(inputs["Wc"], np.float32)
    gn_w = np.asarray(inputs["gn_w"], np.float32)
    gn_b = np.asarray(inputs["gn_b"], np.float32)
    gamma = np.asarray(inputs["gamma"], np.float32)

    def sig(v):
        return 1.0 / (1.0 + np.exp(-v))

    lam = (sig(np.asarray(inputs["lq1"], np.float32).reshape(H)
               * np.asarray(inputs["lk1"], np.float32).reshape(H))
           - sig(np.asarray(inputs["lq2"], np.float32).reshape(H)
                 * np.asarray(inputs["lk2"], np.float32).reshape(H))
           + LAMBDA_INIT)

    mask = np.where(np.arange(128)[None, :] <= np.arange(128)[:, None],
                    0.0, NEG).astype(bf)
    ident = np.eye(128, dtype=np.float32).astype(bf)
    gg = (np.arange(128)[:, None] // 4 == np.arange(128)[None, :] // 4
          ).astype(np.float32)
    c1 = 1.0 - LAMBDA_INIT
    gw2 = (gn_w * gamma * c1).astype(np.float32).reshape(128, 1)
    gb2 = (gn_b * gamma * c1).astype(np.float32).reshape(128, 1)

    icon = np.zeros((128, 2), np.uint32)
    icon[:, 0] = 1
    icon[:, 1] = 0x5f375a00
    xTb = [np.ascontiguousarray(x[b].T).astype(bf) for b in range(B)]
    in_maps = []
    for core in range(N_CORES):
        b, hg = divmod(core, N_CORES // B)
        qs = hg * NH * HS          # 256-wide q/k slice
        vs = hg * NH * D           # 512-wide v / y2 slice
        lamn = np.repeat(-lam[hg * NH:(hg + 1) * NH].reshape(1, NH),
                         128, axis=0).astype(np.float32)
        def mtiles(wt, cols):
            # [1024, cols] -> merged 2-tile layout [256, 4*cols]
            return np.ascontiguousarray(
                wt.reshape(2, 4, 128, cols).transpose(0, 2, 1, 3)
                .reshape(256, 4 * cols)).astype(bf)

        in_maps.append({
            "xT": xTb[b],
            "wq1T": mtiles(Wq1[qs:qs + NH * HS, :].T, NH * HS),
            "wq2T": mtiles(Wq2[qs:qs + NH * HS, :].T, NH * HS),
            "wk1T": mtiles(Wk1[qs:qs + NH * HS, :].T, NH * HS),
            "wk2T": mtiles(Wk2[qs:qs + NH * HS, :].T, NH * HS),
            "wvT": mtiles(Wv[vs:vs + NH * D, :].T, NH * D),
            "wcT": np.ascontiguousarray(Wc[:, vs:vs + NH * D].T).astype(bf),
            "mask128": mask,
            "ident": ident,
            "gg": gg,
            "gw2": gw2,
            "gb2": gb2,
            "lamn": lamn,
            "icon": icon,
        })
    return in_maps


def kernel(**inputs):
    if "nc" not in _cache:
        _cache["nc"] = _build()
    nc = _cache["nc"]
    in_maps = _prep_inputs(inputs)
    res = bass_utils.run_bass_kernel_spmd(
        nc, in_maps, core_ids=list(range(N_CORES)),
        **_cache.get("run_kwargs", {}))
    _cache["last_result"] = res
    out = np.zeros((B, T, C), np.float32)
    for core in range(N_CORES):
        b = core // (N_CORES // B)
        out[b] += res.results[core]["outT"].T
    return out



# revision 10
# speedup vs baseline: 1.0054x; 1.0054x over previous
"""Trainium2 Bass kernel for nn_MultiHeadDiffAttention (B=2,T=2048,C=1024,H=16).

Sharding: 8 cores = data-parallel over B(2) x tensor-parallel over 4 head-groups
(4 heads each). Each core computes q/k/v projections for its heads, causal
differential attention, per-head GroupNorm, and a partial output projection
(its 512 columns of y2 against Wc). Host sums the 4 partials per batch.

v3 schedule (over the v2 layout):
  - x DMA'd first across all 4 issue queues; q1/k1 then q2/k2 weights next,
    so attention S-matmuls start ~30us in instead of ~60us.
  - only the oc=0 (heads 0-1) q/k projections run up front; oc=1 and the v
    projection drain as background PE work inside the head-0/1 loops.
  - head pair (2,3) has no projection work left to fill PE gaps (its phase
    is ACT-exp-bound), so the j=0/1 half of the output projection runs
    there instead: partial (y0*Wc0 + y1*Wc1) per ocb into bf16 SBUF tiles
    (space freed by closing the loads pool at the pair boundary), added
    back in the final phase via an identity-stationary matmul.
  - combine att' = e1 + s*e2 split into tensor_scalar (4x DVE mode) +
    tensor_tensor (2x) instead of one 1x scalar_tensor_tensor pass.
  - psum->sbuf evictions rotate across DVE/ACT/GPSIMD so neither ACT (exp)
    nor DVE (combine/casts) eats them all.
  - outT is DMA'd in bf16 (halves the output traffic); host upcasts.
"""

import sys

for _p in ("/opt/trn_rl_repo", "/root/.axon_site/_ro/trn_rl_repo"):
    if _p not in sys.path:
        sys.path.insert(0, _p)

import math
import numpy as np
import ml_dtypes

import concourse.bass as bass
import concourse.bacc as bacc
import concourse.tile as tile
import concourse.mybir as mybir
from concourse import bass_utils

F32 = mybir.dt.float32
BF16 = mybir.dt.bfloat16
AF = mybir.ActivationFunctionType
ALU = mybir.AluOpType

B, T, C = 2, 2048, 1024
H = 16
HS = C // H           # 64
D = 2 * HS            # 128 v-channels per head
NH = 4                # heads per core
N_CORES = 8
NT = T // 128         # 16 q-tiles
LAMBDA_INIT = 0.8 - 0.6 * math.exp(-0.3 * (12 - 1))
EPS = 1e-5
SCALE = 1.0 / math.sqrt(HS)
NEG = -30000.0

_cache = {}


def _build(T=T, trace_sim=False, nh=NH):
    NT = T // 128
    nc = bacc.Bacc("TRN2", target_bir_lowering=False, debug=False,
                   num_devices=N_CORES)

    def din(name, shape, dt=BF16):
        return nc.dram_tensor(name, shape, dt, kind="ExternalInput").ap()

    xT_d = din("xT", [C, T])
    # q/k/v weights arrive in host-merged tile layout: [2 tiles x 128
    # partitions, 4 chunks x cols] so each needs only a few DMAs
    wq1_d = din("wq1T", [256, 4 * NH * HS])
    wq2_d = din("wq2T", [256, 4 * NH * HS])
    wk1_d = din("wk1T", [256, 4 * NH * HS])
    wk2_d = din("wk2T", [256, 4 * NH * HS])
    wv_d = din("wvT", [256, 4 * NH * D])
    wc_d = din("wcT", [NH * D, C])
    mask_d = din("mask128", [128, 128])
    ident_d = din("ident", [128, 128])
    gg_d = din("gg", [128, 128], F32)
    gw2_d = din("gw2", [128, 1], F32)
    gb2_d = din("gb2", [128, 1], F32)
    lamn_d = din("lamn", [128, NH], F32)
    icon_d = din("icon", [128, 2], mybir.dt.uint32)
    outT_d = nc.dram_tensor("outT", [C, T], BF16, kind="ExternalOutput").ap()

    with tile.TileContext(nc, trace_sim=trace_sim) as tc:
        with tc.tile_pool(name="persist", bufs=1) as pp, \
             tc.tile_pool(name="ps_s", bufs=3, space="PSUM") as ps_s, \
             tc.tile_pool(name="ps_t", bufs=2, space="PSUM") as ps_t:

            # ---- PE warmup: ramp the tensor-engine P-state during the
            # input DMA window with dummy back-to-back matmuls ----
            wa_t = pp.tile([128, 256], BF16, tag="wa")
            nc.vector.memset(wa_t[:], 0.0)
            for _ in range(48):
                psw = ps_s.tile([128, 1024], F32, tag="s")
                nc.tensor.matmul(psw[:, 0:256], wa_t[:, 0:128], wa_t[:],
                                 start=True, stop=True)

            # ---- persistent small tiles ----
            mask_t = pp.tile([128, 128], BF16, tag="mask")
            ident_t = pp.tile([128, 128], BF16, tag="ident")
            gg_t = pp.tile([128, 128], F32, tag="gg")
            gw2_t = pp.tile([128, 1], F32, tag="gw2")
            gb2_t = pp.tile([128, 1], F32, tag="gb2")
            lamn_t = pp.tile([128, NH], F32, tag="lamn")
            c15_t = pp.tile([128, 1], F32, tag="c15")
            nc.vector.memset(c15_t[:], 1.5)
            icon_t = pp.tile([128, 2], mybir.dt.uint32, tag="icon")

            # ---- persistent activation tensors ----
            q1t = [pp.tile([128, T], BF16, tag=f"q1t{i}", name=f"q1t{i}") for i in range(2)]
            q2t = [pp.tile([128, T], BF16, tag=f"q2t{i}", name=f"q2t{i}") for i in range(2)]
            k1t = [pp.tile([128, T], BF16, tag=f"k1t{i}", name=f"k1t{i}") for i in range(2)]
            k2t = [pp.tile([128, T], BF16, tag=f"k2t{i}", name=f"k2t{i}") for i in range(2)]
            vt = [pp.tile([128, NH * D], BF16, tag=f"vt{i}", name=f"vt{i}") for i in range(NT)]
            # yT per head [D=128, T] bf16 (post-groupnorm)
            yt = [pp.tile([128, T], BF16, tag=f"yt{j}", name=f"yt{j}") for j in range(NH)]
            # wcT: [512, C] as 4 f-chunk tiles (one per head)
            wct = [pp.tile([128, C], BF16, tag=f"wct{j}", name=f"wct{j}") for j in range(NH)]

            # attention working pool opened BEFORE the loads pool so the
            # loads pool can be released first (pools close LIFO)
            wp_cm = tc.tile_pool(name="aw", bufs=2)
            wp = wp_cm.__enter__()

            # ================= q/k projection loads =================
            lp_cm = tc.tile_pool(name="loads", bufs=1)
            lp = lp_cm.__enter__()
            xt = [lp.tile([128, T], BF16, tag=f"xt{i}", name=f"xt{i}") for i in range(8)]
            wqm = {}
            for nm in ("q1", "q2", "k1", "k2"):
                wqm[nm] = [lp.tile([128, 4 * NH * HS], BF16,
                                   tag=f"w{nm}{h}", name=f"w{nm}{h}")
                           for h in range(2)]
            wvm = [lp.tile([128, 4 * NH * D], BF16, tag=f"wvm{h}",
                           name=f"wvm{h}") for h in range(2)]

            def wq_sl(nm, cc, oc):
                return wqm[nm][cc // 4][:, (cc % 4) * 256 + oc * 128:
                                        (cc % 4) * 256 + oc * 128 + 128]

            def wv_sl(cc):
                return wvm[cc // 4][:, (cc % 4) * 512:(cc % 4) * 512 + 512]

            # ---- DMA schedule (3 issue queues: SP/ACT/Pool): x first,
            # then q1/k1 weights, then q2/k2, then wv/wc + small tiles ----
            queues = [nc.sync, nc.scalar, nc.gpsimd]
            for i in range(8):
                queues[i % 3].dma_start(xt[i][:], xT_d[i * 128:(i + 1) * 128, :])
            # small tiles needed early in attention (tiny; on the queue
            # with one fewer x chunk)
            nc.gpsimd.dma_start(mask_t[:], mask_d)
            nc.gpsimd.dma_start(ident_t[:], ident_d)
            nc.gpsimd.dma_start(lamn_t[:], lamn_d)
            for nm, d_ap, eng in (("q1", wq1_d, nc.sync),
                                  ("k1", wk1_d, nc.scalar),
                                  ("q2", wq2_d, nc.gpsimd),
                                  ("k2", wk2_d, nc.gpsimd)):
                for h in range(2):
                    for cf in range(2):
                        eng.dma_start(
                            wqm[nm][h][:, cf * 512:(cf + 1) * 512],
                            d_ap[h * 128:(h + 1) * 128,
                                 cf * 512:(cf + 1) * 512])
            # wv needed by the first background v chunks (~35us in)
            for h in range(2):
                for cf in range(2):
                    eng = nc.sync if cf == 0 else nc.scalar
                    eng.dma_start(
                        wvm[h][:, cf * 1024:(cf + 1) * 1024],
                        wv_d[h * 128:(h + 1) * 128,
                             cf * 1024:(cf + 1) * 1024])
            # wc + groupnorm consts needed late
            for j in range(NH):
                eng = (nc.sync, nc.scalar)[j % 2]
                eng.dma_start(wct[j][:], wc_d[j * 128:(j + 1) * 128, :])
            nc.gpsimd.dma_start(gg_t[:], gg_d)
            nc.gpsimd.dma_start(gw2_t[:], gw2_d)
            nc.gpsimd.dma_start(gb2_t[:], gb2_d)
            nc.gpsimd.dma_start(icon_t[:], icon_d)

            # qT/kT projections: out [o=128, t=512] = W^T_chunk.T @ xT
            _qk_ps = {}

            def emit_qk(nm, dst, oc, half):
                if half == 0:
                    _qk_ps[(nm, oc)] = (ps_s.tile([128, 1024], F32, tag="s", name="qkA"),
                                        ps_s.tile([128, 1024], F32, tag="s", name="qkB"))
                psA, psB = _qk_ps[(nm, oc)]
                for cc in range(4 * half, 4 * half + 4):
                    for tb in range(T // 512):
                        ph = (psA, psB)[tb // 2]
                        mi = nc.tensor.matmul(
                            ph[:, (tb % 2) * 512:(tb % 2) * 512 + 512],
                            wq_sl(nm, cc, oc),
                            xt[cc][:, tb * 512:(tb + 1) * 512],
                            start=(cc == 0), stop=(cc == 7),
                            skip_group_check=True)
                        if tb > 0:
                            mi.ins.ldweights = False
                if half == 1:
                    for tb in range(T // 512):
                        ph = (psA, psB)[tb // 2]
                        src = ph[:, (tb % 2) * 512:(tb % 2) * 512 + 512]
                        if nm in ("q1", "k1"):
                            nc.scalar.copy(
                                dst[oc][:, tb * 512:(tb + 1) * 512], src)
                        else:
                            nc.vector.tensor_copy(
                                dst[oc][:, tb * 512:(tb + 1) * 512], src)

            def emit_qk_full(nm, dst, oc):
                emit_qk(nm, dst, oc, 0)
                emit_qk(nm, dst, oc, 1)

            # oc=0 projections up front (heads 0-1 attention needs them)
            for nm, dst in (("q1", q1t), ("k1", k1t), ("q2", q2t), ("k2", k2t)):
                emit_qk_full(nm, dst, 0)

            # v projection chunks are emitted lazily
            v_done = [False] * NT

            def emit_v(tch):
                if v_done[tch]:
                    return
                v_done[tch] = True
                ps = ps_t.tile([128, NH * D], F32, tag="tz")
                for cc in range(8):
                    nc.tensor.matmul(
                        ps[:],
                        xt[cc][:, tch * 128:(tch + 1) * 128],
                        wv_sl(cc),
                        start=(cc == 0), stop=(cc == 7))
                if tch % 2 == 0:
                    nc.vector.tensor_copy(vt[tch][:], ps[:])
                else:
                    nc.scalar.copy(vt[tch][:], ps[:])

            # background PE work drained between attention phases of
            # heads 0-1: v chunks just-in-time, oc=1 projections between
            bg = [(lambda t=t: emit_v(t)) for t in range(4)]
            for i, (nm, dst) in enumerate((("q1", q1t), ("k1", k1t),
                                           ("q2", q2t), ("k2", k2t))):
                bg.append(lambda nm=nm, dst=dst: emit_qk_full(nm, dst, 1))
                bg.extend([(lambda t=t: emit_v(t))
                           for t in range(4 + 3 * i, 7 + 3 * i)])

            def bg_drain(n):
                for _ in range(min(n, len(bg))):
                    bg.pop(0)()

            # ================= attention per head =================
            head_ctx = {}

            def phase1(j, qb):
                oc, po = divmod(j * HS, 128)
                att_rows = {}
                for qq in range(4):
                    qt = qb * 4 + qq
                    nk = qt + 1
                    nkb2 = (nk + 7) // 8   # 1024-wide S psum tiles
                    e1 = wp.tile([128, T], BF16, tag="e1", name="e1", bufs=8)
                    e2 = wp.tile([128, T], BF16, tag="e2", name="e2", bufs=2)
                    d1c = wp.tile([128, 2], F32, tag="d1c", name="d1c")
                    d2c = wp.tile([128, 2], F32, tag="d2c", name="d2c")
                    for mi, (qsrc, ksrc, erow, dcol) in enumerate(
                            ((q1t, k1t, e1, d1c), (q2t, k2t, e2, d2c))):
                        for kb in range(nkb2):
                            w = min(1024, nk * 128 - kb * 1024)
                            ps = ps_s.tile([128, 1024], F32, tag="s",
                                           name="ps")
                            off = qt * 128 - kb * 1024  # diag block col
                            for hf in range(2):
                                wh = min(512, w - hf * 512)
                                if wh <= 0:
                                    break
                                diag_here = (kb == nkb2 - 1 and
                                             hf * 512 <= off < hf * 512 + wh)
                                mi = nc.tensor.matmul(
                                    ps[:, hf * 512:hf * 512 + wh],
                                    qsrc[oc][po:po + HS,
                                             qt * 128:(qt + 1) * 128],
                                    ksrc[oc][po:po + HS,
                                             kb * 1024 + hf * 512:
                                             kb * 1024 + hf * 512 + wh],
                                    start=True, stop=not diag_here,
                                    skip_group_check=diag_here)
                                if kb + hf > 0:
                                    mi.ins.ldweights = False
                            if kb == nkb2 - 1:
                                # mask diagonal 128-block on PE
                                nc.tensor.matmul(
                                    ps[:, off:off + 128],
                                    ident_t[:], mask_t[:],
                                    start=False, stop=True,
                                    skip_group_check=True)
                            nc.scalar.activation(
                                erow[:, kb * 1024:kb * 1024 + w],
                                ps[:, :w], AF.Exp, scale=SCALE,
                                accum_out=dcol[:, kb:kb + 1])
                    # denominators -> rr = [1/D1, 1/D2]
                    rr = wp.tile([128, 2], F32, tag="rr", name="rr")
                    dd = wp.tile([128, 2], F32, tag="dd", name="dd")
                    if nkb2 == 1:
                        nc.vector.tensor_copy(dd[:, 0:1], d1c[:, 0:1])
                        nc.vector.tensor_copy(dd[:, 1:2], d2c[:, 0:1])
                    else:
                        nc.vector.tensor_reduce(
                            dd[:, 0:1], d1c[:, 0:nkb2],
                            axis=mybir.AxisListType.X, op=ALU.add)
                        nc.vector.tensor_reduce(
                            dd[:, 1:2], d2c[:, 0:nkb2],
                            axis=mybir.AxisListType.X, op=ALU.add)
                    nc.vector.reciprocal(rr[:], dd[:, 0:2])
                    # s = -lam * D1 / D2 (per-partition scalar)
                    sc = wp.tile([128, 1], F32, tag="sc", name="sc")
                    nc.vector.scalar_tensor_tensor(
                        sc[:], dd[:, 0:1], lamn_t[:, j:j + 1], rr[:, 1:2],
                        op0=ALU.mult, op1=ALU.mult)
                    # diag(r1) bf16 for the normalizing transpose
                    dg = wp.tile([128, 128], BF16, tag="dg", name="dg",
                                 bufs=8)
                    nc.vector.tensor_scalar(dg[:], ident_t[:], rr[:, 0:1],
                                            None, op0=ALU.mult)
                    # att' = e1 + s*e2: tensor_scalar (4x) + tensor_tensor
                    # (2x) beats one scalar_tensor_tensor pass (1x)
                    etmp = wp.tile([128, T], BF16, tag="etmp", name="etmp",
                                   bufs=1)
                    nc.vector.tensor_scalar(
                        etmp[:, :nk * 128], e2[:, :nk * 128], sc[:, 0:1],
                        None, op0=ALU.mult)
                    nc.vector.tensor_tensor(
                        e1[:, :nk * 128], e1[:, :nk * 128],
                        etmp[:, :nk * 128], ALU.add)
                    att_rows[qt] = (e1, dg)
                return att_rows

            _cast_rr = [0]

            def phase2(j, qb, att_rows):
                ytr, s1p, s2p = head_ctx[j]
                nkc = qb * 4 + 4
                ablk = []
                for kc in range(nkc):
                    emit_v(kc)
                    pt = ps_t.tile([128, 512], F32, tag="tz", name="pt")
                    zw = max(0, (kc - qb * 4) * 128)
                    for qq in range(4):
                        qt = qb * 4 + qq
                        if qt >= kc:
                            att, dg = att_rows[qt]
                            nc.tensor.matmul(
                                pt[:, qq * 128:qq * 128 + 128],
                                att[:, kc * 128:kc * 128 + 128],
                                dg[:],
                                start=True, stop=True,
                                skip_group_check=True)
                    ab = wp.tile([128, 512], BF16, tag=f"atb{kc}",
                                 name=f"atb{kc}", bufs=1)
                    r = _cast_rr[0] = (_cast_rr[0] + 1) % 3
                    if r == 0:
                        nc.scalar.copy(ab[:, zw:], pt[:, zw:])
                    else:
                        nc.vector.tensor_copy(ab[:, zw:], pt[:, zw:])
                    ablk.append((ab, zw))
                # yT[d, qblk] = sum_kc v_kc.T @ attT_kc   (N=512)
                py = ps_t.tile([128, 512], F32, tag="tz", name="py")
                for kc in range(nkc):
                    ab, zw = ablk[kc]
                    nc.tensor.matmul(
                        py[:, zw:],
                        vt[kc][:, j * 128:(j + 1) * 128],
                        ab[:, zw:],
                        start=(kc == 0), stop=(kc == nkc - 1),
                        skip_group_check=True)
                # copy to ytr with fused stats accumulation (ACT)
                nc.scalar.activation(
                    ytr[:, qb * 512:(qb + 1) * 512], py[:], AF.Copy,
                    accum_out=s1p[:, qb:qb + 1])
                ysq = wp.tile([128, 512], BF16, tag="ysq", name="ysq",
                              bufs=1)
                nc.scalar.activation(
                    ysq[:], py[:], AF.Square,
                    accum_out=s2p[:, qb:qb + 1])

            def gn_final(j):
                ytr, s1p, s2p = head_ctx[j]
                s12 = wp.tile([128, 2], F32, tag="s12", name="s12")
                nc.vector.tensor_reduce(s12[:, 0:1], s1p[:, 0:NT // 4],
                                        axis=mybir.AxisListType.X, op=ALU.add)
                nc.vector.tensor_reduce(s12[:, 1:2], s2p[:, 0:NT // 4],
                                        axis=mybir.AxisListType.X, op=ALU.add)
                pg = ps_t.tile([128, 2], F32, tag="tz", name="pg")
                nc.tensor.matmul(pg[:], gg_t[:], s12[:], start=True, stop=True)
                # mneg = -mean; nvar = mean^2 - E[y^2] = -var
                mneg = wp.tile([128, 1], F32, tag="mneg", name="mneg")
                nc.scalar.mul(mneg[:], pg[:, 0:1], -1.0 / (T * 4))
                msq = wp.tile([128, 1], F32, tag="msq")
                nc.scalar.mul(msq[:], pg[:, 1:2], 1.0 / (T * 4))
                nvar = wp.tile([128, 1], F32, tag="nvar")
                nc.vector.scalar_tensor_tensor(
                    nvar[:], mneg[:], mneg[:, 0:1], msq[:],
                    op0=ALU.mult, op1=ALU.subtract)
                vpe = wp.tile([128, 1], F32, tag="vpe")
                nc.vector.tensor_scalar(vpe[:], nvar[:], -1.0, EPS,
                                        op0=ALU.mult, op1=ALU.add)  # var+eps
                # rsqrt(var+eps) on DVE only: quake seed + Newton iters
                rstd = wp.tile([128, 1], F32, tag="rstd")
                yi = wp.tile([128, 1], F32, tag="yi")
                nc.vector.tensor_tensor(yi.bitcast(mybir.dt.uint32)[:],
                                        vpe.bitcast(mybir.dt.uint32)[:],
                                        icon_t[:, 0:1],
                                        ALU.logical_shift_right)
                nc.vector.tensor_tensor(yi.bitcast(mybir.dt.uint32)[:],
                                        icon_t[:, 1:2],
                                        yi.bitcast(mybir.dt.uint32)[:],
                                        ALU.subtract)
                vneg = wp.tile([128, 1], F32, tag="vneg")
                nc.vector.tensor_scalar_mul(vneg[:], vpe[:], -0.5)
                ytmp = wp.tile([128, 1], F32, tag="ytmp")
                for _ in range(3):
                    nc.vector.tensor_tensor(ytmp[:], yi[:], yi[:], ALU.mult)
                    nc.vector.scalar_tensor_tensor(
                        ytmp[:], ytmp[:], vneg[:, 0:1], c15_t[:],
                        op0=ALU.mult, op1=ALU.add)  # 1.5 - 0.5 v y^2
                    nc.vector.tensor_tensor(yi[:], yi[:], ytmp[:], ALU.mult)
                nc.vector.tensor_copy(rstd[:], yi[:])
                aff_a = wp.tile([128, 1], F32, tag="aff_a")
                nc.vector.tensor_tensor(aff_a[:], rstd[:], gw2_t[:], ALU.mult)
                aff_b = wp.tile([128, 1], F32, tag="aff_b")
                nc.vector.scalar_tensor_tensor(
                    aff_b[:], mneg[:], aff_a[:, 0:1], gb2_t[:],
                    op0=ALU.mult, op1=ALU.add)  # gb2 - mean*aff_a
                # affine on DVE (4x): yt = ytr*aff_a + aff_b
                nc.vector.tensor_scalar(yt[j][:], ytr[:], aff_a[:, 0:1],
                                        aff_b[:, 0:1],
                                        op0=ALU.mult, op1=ALU.add)

            # driver: software-pipelined, phase2 lags phase1 by LAG q-blocks
            LAG = 1
            pend = []
            gn_pend = []

            def drain_one(defer_gn=True):
                if gn_pend and gn_pend[0][1] <= 0:
                    gn_final(gn_pend.pop(0)[0])
                for i in range(len(gn_pend)):
                    gn_pend[i][1] -= 1
                u = pend.pop(0)
                phase2(*u)
                if u[1] == NT // 4 - 1:
                    if defer_gn:
                        gn_pend.append([u[0], 2])
                    else:
                        gn_final(u[0])

            def new_head(j):
                head_ctx[j] = (
                    wp.tile([128, T], BF16, tag="ytr", name="ytr", bufs=3),
                    wp.tile([128, 4], F32, tag="s1p", name="s1p"),
                    wp.tile([128, 4], F32, tag="s2p", name="s2p"))

            # ---- pair (0,1): bg (v + oc1 projections) fills the PE ----
            for qb in range(NT // 4):
                for j in (0, 1):
                    if qb == 0:
                        new_head(j)
                    att_rows = phase1(j, qb)
                    bg_drain(3)
                    pend.append((j, qb, att_rows))
                    if len(pend) > LAG:
                        drain_one()
            bg_drain(len(bg))

            # pair boundary: release x/weight tiles, open the partial pool
            lp_cm.__exit__(None, None, None)
            dp_cm = tc.tile_pool(name="drain", bufs=1)
            dp = dp_cm.__enter__()
            p01 = [dp.tile([128, T], BF16, tag=f"p01_{ocb}",
                           name=f"p01_{ocb}") for ocb in range(8)]

            # out-proj j=0,1 partials: fill PE during the ACT-bound
            # (2,3) pair. Emitted one ocb per (j,qb) slot.
            fill_q = list(range(8))
            _fill_rr = [0]

            def emit_fill(ocb):
                for tb in range(T // 512):
                    pt = ps_t.tile([128, 512], F32, tag="tz", name="fl")
                    for j in (0, 1):
                        nc.tensor.matmul(
                            pt[:],
                            wct[j][:, ocb * 128:(ocb + 1) * 128],
                            yt[j][:, tb * 512:(tb + 1) * 512],
                            start=(j == 0), stop=(j == 1),
                            skip_group_check=True)
                    nc.vector.tensor_copy(
                        p01[ocb][:, tb * 512:(tb + 1) * 512], pt[:])

            # ---- pair (2,3): fills + attention ----
            first23 = True
            for qb in range(NT // 4):
                for j in (2, 3):
                    if qb == 0:
                        new_head(j)
                    att_rows = phase1(j, qb)
                    pend.append((j, qb, att_rows))
                    if len(pend) > LAG:
                        drain_one()
                    if first23:
                        # fills (and head-3's ytr ring slot) need yt0/yt1:
                        # force gn(0)/gn(1) now; their pg-matmul waits hide
                        # behind the phase2(1,3) transposes just emitted
                        while gn_pend:
                            gn_final(gn_pend.pop(0)[0])
                        first23 = False
                    elif fill_q:
                        emit_fill(fill_q.pop(0))
            while pend:
                drain_one()
            while fill_q:
                emit_fill(fill_q.pop(0))
            while len(gn_pend) > 1:
                gn_final(gn_pend.pop(0)[0])

            # ================= output projection =================
            # per ocb: ident-add of the j01 partial, then j=2, then j=3.
            # gn(3) is emitted just before ocb0 so its ACT->DVE chain hides
            # behind the gn-independent ident-adds + j=2 matmuls.
            def s_halves():
                psA = ps_s.tile([128, 1024], F32, tag="s", name="psA")
                psB = ps_s.tile([128, 1024], F32, tag="s", name="psB")
                return [(psA, 0), (psA, 512), (psB, 0), (psB, 512)]

            def op_add(ocb, halves):
                for tb in range(T // 512):
                    ph, off = halves[tb]
                    mi = nc.tensor.matmul(
                        ph[:, off:off + 512],
                        ident_t[:],
                        p01[ocb][:, tb * 512:(tb + 1) * 512],
                        start=True, stop=False,
                        skip_group_check=True)
                    if tb > 0:
                        mi.ins.ldweights = False

            def op_mms23(ocb, halves):
                for j in (2, 3):
                    for tb in range(T // 512):
                        ph, off = halves[tb]
                        mi = nc.tensor.matmul(
                            ph[:, off:off + 512],
                            wct[j][:, ocb * 128:(ocb + 1) * 128],
                            yt[j][:, tb * 512:(tb + 1) * 512],
                            start=False, stop=(j == 3),
                            skip_group_check=True)
                        if tb > 0:
                            mi.ins.ldweights = False

            _fin_rr = [0]

            def op_fin(ocb, halves):
                for tb in range(T // 512):
                    ph, off = halves[tb]
                    ob = dp.tile([128, 512], BF16, tag="ob", bufs=8,
                                 name="ob")
                    r = _fin_rr[0] = (_fin_rr[0] + 1) % 2
                    if r == 0:
                        nc.vector.tensor_copy(ob[:], ph[:, off:off + 512])
                    else:
                        nc.scalar.copy(ob[:], ph[:, off:off + 512])
                    eng = (nc.sync, nc.gpsimd, nc.scalar)[tb % 3]
                    eng.dma_start(
                        outT_d[ocb * 128:(ocb + 1) * 128,
                               tb * 512:(tb + 1) * 512], ob[:])

            while gn_pend:
                gn_final(gn_pend.pop(0)[0])
            for ocb in range(8):
                halves = s_halves()
                op_add(ocb, halves)
                op_mms23(ocb, halves)
                op_fin(ocb, halves)
            dp_cm.__exit__(None, None, None)
            wp_cm.__exit__(None, None, None)

    nc.compile()
    return nc


def _prep_inputs(inputs):
    bf = ml_dtypes.bfloat16
    x = np.asarray(inputs["x"], np.float32)
    Wq1 = np.asarray(inputs["Wq1"], np.float32)
    Wq2 = np.asarray(inputs["Wq2"], np.float32)
    Wk1 = np.asarray(inputs["Wk1"], np.float32)
    Wk2 = np.asarray(inputs["Wk2"], np.float32)
    Wv = np.asarray(inputs["Wv"], np.float32)
    Wc = np.asarray(inputs["Wc"], np.float32)
    gn_w = np.asarray(inputs["gn_w"], np.float32)
    gn_b = np.asarray(inputs["gn_b"], np.float32)
    gamma = np.asarray(inputs["gamma"], np.float32)

    def sig(v):
        return 1.0 / (1.0 + np.exp(-v))

    lam = (sig(np.asarray(inputs["lq1"], np.float32).reshape(H)
               * np.asarray(inputs["lk1"], np.float32).reshape(H))
           - sig(np.asarray(inputs["lq2"], np.float32).reshape(H)
                 * np.asarray(inputs["lk2"], np.float32).reshape(H))
           + LAMBDA_INIT)

    mask = np.where(np.arange(128)[None, :] <= np.arange(128)[:, None],
                    0.0, NEG).astype(bf)
    ident = np.eye(128, dtype=np.float32).astype(bf)
    gg = (np.arange(128)[:, None] // 4 == np.arange(128)[None, :] // 4
          ).astype(np.float32)
    c1 = 1.0 - LAMBDA_INIT
    gw2 = (gn_w * gamma * c1).astype(np.float32).reshape(128, 1)
    gb2 = (gn_b * gamma * c1).astype(np.float32).reshape(128, 1)

    icon = np.zeros((128, 2), np.uint32)
    icon[:, 0] = 1
    icon[:, 1] = 0x5f375a00
    xTb = [np.ascontiguousarray(x[b].T).astype(bf) for b in range(B)]
    in_maps = []
    for core in range(N_CORES):
        b, hg = divmod(core, N_CORES // B)
        qs = hg * NH * HS          # 256-wide q/k slice
        vs = hg * NH * D           # 512-wide v / y2 slice
        lamn = np.repeat(-lam[hg * NH:(hg + 1) * NH].reshape(1, NH),
                         128, axis=0).astype(np.float32)
        def mtiles(wt, cols):
            # [1024, cols] -> merged 2-tile layout [256, 4*cols]
            return np.ascontiguousarray(
                wt.reshape(2, 4, 128, cols).transpose(0, 2, 1, 3)
                .reshape(256, 4 * cols)).astype(bf)

        in_maps.append({
            "xT": xTb[b],
            "wq1T": mtiles(Wq1[qs:qs + NH * HS, :].T, NH * HS),
            "wq2T": mtiles(Wq2[qs:qs + NH * HS, :].T, NH * HS),
            "wk1T": mtiles(Wk1[qs:qs + NH * HS, :].T, NH * HS),
            "wk2T": mtiles(Wk2[qs:qs + NH * HS, :].T, NH * HS),
            "wvT": mtiles(Wv[vs:vs + NH * D, :].T, NH * D),
            "wcT": np.ascontiguousarray(Wc[:, vs:vs + NH * D].T).astype(bf),
            "mask128": mask,
            "ident": ident,
            "gg": gg,
            "gw2": gw2,
            "gb2": gb2,
            "lamn": lamn,
            "icon": icon,
        })
    return in_maps


def kernel(**inputs):
    if "nc" not in _cache:
        _cache["nc"] = _build()
    nc = _cache["nc"]
    in_maps = _prep_inputs(inputs)
    res = bass_utils.run_bass_kernel_spmd(
        nc, in_maps, core_ids=list(range(N_CORES)),
        **_cache.get("run_kwargs", {}))
    _cache["last_result"] = res
    out = np.zeros((B, T, C), np.float32)
    for core in range(N_CORES):
        b = core // (N_CORES // B)
        out[b] += res.results[core]["outT"].T.astype(np.float32)
    return out


# revision 17
# speedup vs baseline: 1.1063x; 1.1004x over previous
"""Trainium2 Bass kernel for nn_MultiHeadDiffAttention (B=2,T=2048,C=1024,H=16).

Sharding: 8 cores = data-parallel over B(2) x tensor-parallel over 4 head-groups
(4 heads each). Each core computes q/k/v projections for its heads, causal
differential attention, per-head GroupNorm, and a partial output projection
(its 512 columns of y2 against Wc). Host sums the 4 partials per batch.

v3 schedule (over the v2 layout):
  - x DMA'd first across all 4 issue queues; q1/k1 then q2/k2 weights next,
    so attention S-matmuls start ~30us in instead of ~60us.
  - only the oc=0 (heads 0-1) q/k projections run up front; oc=1 and the v
    projection drain as background PE work inside the head-0/1 loops.
  - head pair (2,3) has no projection work left to fill PE gaps (its phase
    is ACT-exp-bound), so the j=0/1 half of the output projection runs
    there instead: partial (y0*Wc0 + y1*Wc1) per ocb into bf16 SBUF tiles
    (space freed by closing the loads pool at the pair boundary), added
    back in the final phase via an identity-stationary matmul.
  - combine att' = e1 + s*e2 split into tensor_scalar (4x DVE mode) +
    tensor_tensor (2x) instead of one 1x scalar_tensor_tensor pass.
  - psum->sbuf evictions rotate across DVE/ACT/GPSIMD so neither ACT (exp)
    nor DVE (combine/casts) eats them all.
  - outT is DMA'd in bf16 (halves the output traffic); host upcasts.
"""

import sys

for _p in ("/opt/trn_rl_repo", "/root/.axon_site/_ro/trn_rl_repo"):
    if _p not in sys.path:
        sys.path.insert(0, _p)

import math
import numpy as np
import ml_dtypes

import concourse.bass as bass
import concourse.bacc as bacc
import concourse.tile as tile
import concourse.mybir as mybir
from concourse import bass_utils

F32 = mybir.dt.float32
BF16 = mybir.dt.bfloat16
AF = mybir.ActivationFunctionType
ALU = mybir.AluOpType

B, T, C = 2, 2048, 1024
H = 16
HS = C // H           # 64
D = 2 * HS            # 128 v-channels per head
NH = 4                # heads per core
N_CORES = 8
NT = T // 128         # 16 q-tiles
LAMBDA_INIT = 0.8 - 0.6 * math.exp(-0.3 * (12 - 1))
EPS = 1e-5
SCALE = 1.0 / math.sqrt(HS)
NEG = -30000.0

_cache = {}


def _build(T=T, trace_sim=False, nh=NH):
    NT = T // 128
    nc = bacc.Bacc("TRN2", target_bir_lowering=False, debug=False,
                   num_devices=N_CORES)

    def din(name, shape, dt=BF16):
        return nc.dram_tensor(name, shape, dt, kind="ExternalInput").ap()

    xT_d = din("xT", [C, T])
    # q/k/v weights arrive in host-merged tile layout: [2 tiles x 128
    # partitions, 4 chunks x cols] so each needs only a few DMAs
    wq1_d = din("wq1T", [256, 4 * NH * HS])
    wq2_d = din("wq2T", [256, 4 * NH * HS])
    wk1_d = din("wk1T", [256, 4 * NH * HS])
    wk2_d = din("wk2T", [256, 4 * NH * HS])
    wv_d = din("wvT", [256, 4 * NH * D])
    wc_d = din("wcT", [NH * D, C])
    mask_d = din("mask128", [128, 128])
    ident_d = din("ident", [128, 128])
    gg_d = din("gg", [128, 128], F32)
    gw2_d = din("gw2", [128, 1], F32)
    gb2_d = din("gb2", [128, 1], F32)
    lamn_d = din("lamn", [128, NH], F32)
    icon_d = din("icon", [128, 2], mybir.dt.uint32)
    outT_d = nc.dram_tensor("outT", [C, T], BF16, kind="ExternalOutput").ap()

    with tile.TileContext(nc, trace_sim=trace_sim) as tc:
        with tc.tile_pool(name="persist", bufs=1) as pp, \
             tc.tile_pool(name="ps_s", bufs=3, space="PSUM") as ps_s, \
             tc.tile_pool(name="ps_t", bufs=2, space="PSUM") as ps_t:

            # ---- PE warmup: ramp the tensor-engine P-state during the
            # input DMA window with dummy back-to-back matmuls ----
            wa_t = pp.tile([128, 256], BF16, tag="wa")
            nc.vector.memset(wa_t[:], 0.0)
            for _ in range(48):
                psw = ps_s.tile([128, 1024], F32, tag="s")
                nc.tensor.matmul(psw[:, 0:256], wa_t[:, 0:128], wa_t[:],
                                 start=True, stop=True)

            # ---- persistent small tiles ----
            mask_t = pp.tile([128, 128], BF16, tag="mask")
            ident_t = pp.tile([128, 128], BF16, tag="ident")
            gg_t = pp.tile([128, 128], F32, tag="gg")
            gw2_t = pp.tile([128, 1], F32, tag="gw2")
            gb2_t = pp.tile([128, 1], F32, tag="gb2")
            lamn_t = pp.tile([128, NH], F32, tag="lamn")
            c15_t = pp.tile([128, 1], F32, tag="c15")
            nc.vector.memset(c15_t[:], 1.5)
            icon_t = pp.tile([128, 2], mybir.dt.uint32, tag="icon")

            # ---- persistent activation tensors ----
            q1t = [pp.tile([128, T], BF16, tag=f"q1t{i}", name=f"q1t{i}") for i in range(2)]
            q2t = [pp.tile([128, T], BF16, tag=f"q2t{i}", name=f"q2t{i}") for i in range(2)]
            k1t = [pp.tile([128, T], BF16, tag=f"k1t{i}", name=f"k1t{i}") for i in range(2)]
            k2t = [pp.tile([128, T], BF16, tag=f"k2t{i}", name=f"k2t{i}") for i in range(2)]
            vt = [pp.tile([128, NH * D], BF16, tag=f"vt{i}", name=f"vt{i}") for i in range(NT)]
            # yT per head [D=128, T] bf16 (post-groupnorm)
            yt = [pp.tile([128, T], BF16, tag=f"yt{j}", name=f"yt{j}") for j in range(NH)]
            # wcT: [512, C] as 4 f-chunk tiles (one per head)
            wct = [pp.tile([128, C], BF16, tag=f"wct{j}", name=f"wct{j}") for j in range(NH)]

            # attention working pool opened BEFORE the loads pool so the
            # loads pool can be released first (pools close LIFO)
            wp_cm = tc.tile_pool(name="aw", bufs=2)
            wp = wp_cm.__enter__()

            # ================= q/k projection loads =================
            lp_cm = tc.tile_pool(name="loads", bufs=1)
            lp = lp_cm.__enter__()
            xt = [lp.tile([128, T], BF16, tag=f"xt{i}", name=f"xt{i}") for i in range(8)]
            wqm = {}
            for nm in ("q1", "q2", "k1", "k2"):
                wqm[nm] = [lp.tile([128, 4 * NH * HS], BF16,
                                   tag=f"w{nm}{h}", name=f"w{nm}{h}")
                           for h in range(2)]
            wvm = [lp.tile([128, 4 * NH * D], BF16, tag=f"wvm{h}",
                           name=f"wvm{h}") for h in range(2)]

            def wq_sl(nm, cc, oc):
                return wqm[nm][cc // 4][:, (cc % 4) * 256 + oc * 128:
                                        (cc % 4) * 256 + oc * 128 + 128]

            def wv_sl(cc):
                return wvm[cc // 4][:, (cc % 4) * 512:(cc % 4) * 512 + 512]

            # ---- DMA schedule (3 issue queues: SP/ACT/Pool): x first,
            # then q1/k1 weights, then q2/k2, then wv/wc + small tiles ----
            queues = [nc.sync, nc.scalar, nc.gpsimd]
            for i in range(8):
                queues[i % 3].dma_start(xt[i][:], xT_d[i * 128:(i + 1) * 128, :])
            # small tiles needed early in attention (tiny; on the queue
            # with one fewer x chunk)
            nc.gpsimd.dma_start(mask_t[:], mask_d)
            nc.gpsimd.dma_start(ident_t[:], ident_d)
            nc.gpsimd.dma_start(lamn_t[:], lamn_d)
            for nm, d_ap, eng in (("q1", wq1_d, nc.sync),
                                  ("k1", wk1_d, nc.scalar),
                                  ("q2", wq2_d, nc.gpsimd),
                                  ("k2", wk2_d, nc.gpsimd)):
                for h in range(2):
                    for cf in range(2):
                        eng.dma_start(
                            wqm[nm][h][:, cf * 512:(cf + 1) * 512],
                            d_ap[h * 128:(h + 1) * 128,
                                 cf * 512:(cf + 1) * 512])
            # wv needed by the first background v chunks (~35us in)
            for h in range(2):
                for cf in range(2):
                    eng = nc.sync if cf == 0 else nc.scalar
                    eng.dma_start(
                        wvm[h][:, cf * 1024:(cf + 1) * 1024],
                        wv_d[h * 128:(h + 1) * 128,
                             cf * 1024:(cf + 1) * 1024])
            # wc + groupnorm consts needed late
            for j in range(NH):
                eng = (nc.sync, nc.scalar)[j % 2]
                eng.dma_start(wct[j][:], wc_d[j * 128:(j + 1) * 128, :])
            nc.gpsimd.dma_start(gg_t[:], gg_d)
            nc.gpsimd.dma_start(gw2_t[:], gw2_d)
            nc.gpsimd.dma_start(gb2_t[:], gb2_d)
            nc.gpsimd.dma_start(icon_t[:], icon_d)

            # qT/kT projections: out [o=128, t=512] = W^T_chunk.T @ xT
            _qk_ps = {}

            def emit_qk(nm, dst, oc, half):
                if half == 0:
                    _qk_ps[(nm, oc)] = (ps_s.tile([128, 1024], F32, tag="s", name="qkA"),
                                        ps_s.tile([128, 1024], F32, tag="s", name="qkB"))
                psA, psB = _qk_ps[(nm, oc)]
                for cc in range(4 * half, 4 * half + 4):
                    for tb in range(T // 512):
                        ph = (psA, psB)[tb // 2]
                        mi = nc.tensor.matmul(
                            ph[:, (tb % 2) * 512:(tb % 2) * 512 + 512],
                            wq_sl(nm, cc, oc),
                            xt[cc][:, tb * 512:(tb + 1) * 512],
                            start=(cc == 0), stop=(cc == 7),
                            skip_group_check=True)
                        if tb > 0:
                            mi.ins.ldweights = False
                if half == 1:
                    for tb in range(T // 512):
                        ph = (psA, psB)[tb // 2]
                        src = ph[:, (tb % 2) * 512:(tb % 2) * 512 + 512]
                        if nm in ("q1", "k1"):
                            nc.scalar.copy(
                                dst[oc][:, tb * 512:(tb + 1) * 512], src)
                        else:
                            nc.vector.tensor_copy(
                                dst[oc][:, tb * 512:(tb + 1) * 512], src)

            def emit_qk_full(nm, dst, oc):
                emit_qk(nm, dst, oc, 0)
                emit_qk(nm, dst, oc, 1)

            # oc=0 projections up front (heads 0-1 attention needs them)
            for nm, dst in (("q1", q1t), ("k1", k1t), ("q2", q2t), ("k2", k2t)):
                emit_qk_full(nm, dst, 0)

            # v projection chunks are emitted lazily
            v_done = [False] * NT

            def emit_v(tch):
                if v_done[tch]:
                    return
                v_done[tch] = True
                ps = ps_t.tile([128, NH * D], F32, tag="tz")
                for cc in range(8):
                    nc.tensor.matmul(
                        ps[:],
                        xt[cc][:, tch * 128:(tch + 1) * 128],
                        wv_sl(cc),
                        start=(cc == 0), stop=(cc == 7))
                if tch % 2 == 0:
                    nc.vector.tensor_copy(vt[tch][:], ps[:])
                else:
                    nc.scalar.copy(vt[tch][:], ps[:])

            # background PE work drained between attention phases of
            # heads 0-1: v chunks just-in-time, oc=1 projections between
            bg = [(lambda t=t: emit_v(t)) for t in range(4)]
            for i, (nm, dst) in enumerate((("q1", q1t), ("k1", k1t),
                                           ("q2", q2t), ("k2", k2t))):
                bg.append(lambda nm=nm, dst=dst: emit_qk_full(nm, dst, 1))
                bg.extend([(lambda t=t: emit_v(t))
                           for t in range(4 + 3 * i, 7 + 3 * i)])

            def bg_drain(n):
                for _ in range(min(n, len(bg))):
                    bg.pop(0)()

            # ================= attention per head =================
            head_ctx = {}

            def phase1(j, qb):
                oc, po = divmod(j * HS, 128)
                # AB holds the DMA-transposed normalized attention rows for
                # this (j, qb): layout [k=128, kc, qq, q-col] so the z-matmul
                # rhs slices are contiguous (strided moving operands are
                # silently wrong on HW; strided DMA-transpose OUT is fine)
                AB = wp.tile([128, NT, 4, 128], BF16, tag="AB", name="AB",
                             bufs=2)
                for qq in range(4):
                    qt = qb * 4 + qq
                    nk = qt + 1
                    nkb2 = (nk + 7) // 8   # 1024-wide S psum tiles
                    e1 = wp.tile([128, T], BF16, tag="e1", name="e1", bufs=3)
                    e2 = wp.tile([128, T], BF16, tag="e2", name="e2", bufs=2)
                    d1c = wp.tile([128, 2], F32, tag="d1c", name="d1c")
                    d2c = wp.tile([128, 2], F32, tag="d2c", name="d2c")
                    for mi, (qsrc, ksrc, erow, dcol) in enumerate(
                            ((q1t, k1t, e1, d1c), (q2t, k2t, e2, d2c))):
                        for kb in range(nkb2):
                            w = min(1024, nk * 128 - kb * 1024)
                            ps = ps_s.tile([128, 1024], F32, tag="s",
                                           name="ps")
                            off = qt * 128 - kb * 1024  # diag block col
                            for hf in range(2):
                                wh = min(512, w - hf * 512)
                                if wh <= 0:
                                    break
                                diag_here = (kb == nkb2 - 1 and
                                             hf * 512 <= off < hf * 512 + wh)
                                mi = nc.tensor.matmul(
                                    ps[:, hf * 512:hf * 512 + wh],
                                    qsrc[oc][po:po + HS,
                                             qt * 128:(qt + 1) * 128],
                                    ksrc[oc][po:po + HS,
                                             kb * 1024 + hf * 512:
                                             kb * 1024 + hf * 512 + wh],
                                    start=True, stop=not diag_here,
                                    skip_group_check=diag_here)
                                if kb + hf > 0:
                                    mi.ins.ldweights = False
                            if kb == nkb2 - 1:
                                # mask diagonal 128-block on PE
                                nc.tensor.matmul(
                                    ps[:, off:off + 128],
                                    ident_t[:], mask_t[:],
                                    start=False, stop=True,
                                    skip_group_check=True)
                            nc.scalar.activation(
                                erow[:, kb * 1024:kb * 1024 + w],
                                ps[:, :w], AF.Exp, scale=SCALE,
                                accum_out=dcol[:, kb:kb + 1])
                    # denominators -> rr = [1/D1, 1/D2]
                    rr = wp.tile([128, 2], F32, tag="rr", name="rr")
                    dd = wp.tile([128, 2], F32, tag="dd", name="dd")
                    if nkb2 == 1:
                        nc.vector.tensor_copy(dd[:, 0:1], d1c[:, 0:1])
                        nc.vector.tensor_copy(dd[:, 1:2], d2c[:, 0:1])
                    else:
                        nc.vector.tensor_reduce(
                            dd[:, 0:1], d1c[:, 0:nkb2],
                            axis=mybir.AxisListType.X, op=ALU.add)
                        nc.vector.tensor_reduce(
                            dd[:, 1:2], d2c[:, 0:nkb2],
                            axis=mybir.AxisListType.X, op=ALU.add)
                    nc.vector.reciprocal(rr[:], dd[:, 0:2])
                    # sc2 = -lam / D2 (per-partition scalar)
                    sc2 = wp.tile([128, 1], F32, tag="sc2", name="sc2")
                    nc.vector.tensor_tensor(sc2[:], rr[:, 1:2],
                                            lamn_t[:, j:j + 1], ALU.mult)
                    # att = e1*r1 + e2*sc2 (fully normalized diff-attention
                    # row): tensor_scalar (4x) + scalar_tensor_tensor
                    etmp = wp.tile([128, T], BF16, tag="etmp", name="etmp",
                                   bufs=1)
                    nc.vector.tensor_scalar(
                        etmp[:, :nk * 128], e2[:, :nk * 128], sc2[:, 0:1],
                        None, op0=ALU.mult)
                    nc.vector.scalar_tensor_tensor(
                        e1[:, :nk * 128], e1[:, :nk * 128], rr[:, 0:1],
                        etmp[:, :nk * 128], op0=ALU.mult, op1=ALU.add)
                    # transpose this attention row on the DMA xbar:
                    # [q=128, nk*128] -> nk transposed blocks [k=128, 128]
                    nc.sync.dma_start_transpose(
                        AB[:, 0:nk, qq, :], e1[:, :nk * 128])
                return AB

            def phase2(j, qb, AB):
                ytr, s1p, s2p = head_ctx[j]
                nkc = qb * 4 + 4
                # yT[d, qblk] = sum_kc v_kc.T @ attT_kc   (N=512)
                py = ps_t.tile([128, 512], F32, tag="tz", name="py")
                for kc in range(nkc):
                    emit_v(kc)
                    qq0 = max(0, kc - qb * 4)
                    zw = qq0 * 128
                    nc.tensor.matmul(
                        py[:, zw:],
                        vt[kc][:, j * 128:(j + 1) * 128],
                        AB[:, kc, qq0:4, :].rearrange("p q c -> p (q c)"),
                        start=(kc == 0), stop=(kc == nkc - 1),
                        skip_group_check=True)
                # copy to ytr with fused stats accumulation (ACT)
                nc.scalar.activation(
                    ytr[:, qb * 512:(qb + 1) * 512], py[:], AF.Copy,
                    accum_out=s1p[:, qb:qb + 1])
                ysq = wp.tile([128, 512], BF16, tag="ysq", name="ysq",
                              bufs=1)
                nc.scalar.activation(
                    ysq[:], py[:], AF.Square,
                    accum_out=s2p[:, qb:qb + 1])

            def gn_final(j):
                ytr, s1p, s2p = head_ctx[j]
                s12 = wp.tile([128, 2], F32, tag="s12", name="s12")
                nc.vector.tensor_reduce(s12[:, 0:1], s1p[:, 0:NT // 4],
                                        axis=mybir.AxisListType.X, op=ALU.add)
                nc.vector.tensor_reduce(s12[:, 1:2], s2p[:, 0:NT // 4],
                                        axis=mybir.AxisListType.X, op=ALU.add)
                pg = ps_t.tile([128, 2], F32, tag="tz", name="pg")
                nc.tensor.matmul(pg[:], gg_t[:], s12[:], start=True, stop=True)
                # mneg = -mean; nvar = mean^2 - E[y^2] = -var
                mneg = wp.tile([128, 1], F32, tag="mneg", name="mneg")
                nc.scalar.mul(mneg[:], pg[:, 0:1], -1.0 / (T * 4))
                msq = wp.tile([128, 1], F32, tag="msq")
                nc.scalar.mul(msq[:], pg[:, 1:2], 1.0 / (T * 4))
                nvar = wp.tile([128, 1], F32, tag="nvar")
                nc.vector.scalar_tensor_tensor(
                    nvar[:], mneg[:], mneg[:, 0:1], msq[:],
                    op0=ALU.mult, op1=ALU.subtract)
                vpe = wp.tile([128, 1], F32, tag="vpe")
                nc.vector.tensor_scalar(vpe[:], nvar[:], -1.0, EPS,
                                        op0=ALU.mult, op1=ALU.add)  # var+eps
                # rsqrt(var+eps) on DVE only: quake seed + Newton iters
                rstd = wp.tile([128, 1], F32, tag="rstd")
                yi = wp.tile([128, 1], F32, tag="yi")
                nc.vector.tensor_tensor(yi.bitcast(mybir.dt.uint32)[:],
                                        vpe.bitcast(mybir.dt.uint32)[:],
                                        icon_t[:, 0:1],
                                        ALU.logical_shift_right)
                nc.vector.tensor_tensor(yi.bitcast(mybir.dt.uint32)[:],
                                        icon_t[:, 1:2],
                                        yi.bitcast(mybir.dt.uint32)[:],
                                        ALU.subtract)
                vneg = wp.tile([128, 1], F32, tag="vneg")
                nc.vector.tensor_scalar_mul(vneg[:], vpe[:], -0.5)
                ytmp = wp.tile([128, 1], F32, tag="ytmp")
                for _ in range(3):
                    nc.vector.tensor_tensor(ytmp[:], yi[:], yi[:], ALU.mult)
                    nc.vector.scalar_tensor_tensor(
                        ytmp[:], ytmp[:], vneg[:, 0:1], c15_t[:],
                        op0=ALU.mult, op1=ALU.add)  # 1.5 - 0.5 v y^2
                    nc.vector.tensor_tensor(yi[:], yi[:], ytmp[:], ALU.mult)
                nc.vector.tensor_copy(rstd[:], yi[:])
                aff_a = wp.tile([128, 1], F32, tag="aff_a")
                nc.vector.tensor_tensor(aff_a[:], rstd[:], gw2_t[:], ALU.mult)
                aff_b = wp.tile([128, 1], F32, tag="aff_b")
                nc.vector.scalar_tensor_tensor(
                    aff_b[:], mneg[:], aff_a[:, 0:1], gb2_t[:],
                    op0=ALU.mult, op1=ALU.add)  # gb2 - mean*aff_a
                # affine on DVE (4x): yt = ytr*aff_a + aff_b
                nc.vector.tensor_scalar(yt[j][:], ytr[:], aff_a[:, 0:1],
                                        aff_b[:, 0:1],
                                        op0=ALU.mult, op1=ALU.add)

            # driver: software-pipelined, phase2 lags phase1 by LAG q-blocks
            LAG = 1
            pend = []
            gn_pend = []

            def drain_one(defer_gn=True):
                if gn_pend and gn_pend[0][1] <= 0:
                    gn_final(gn_pend.pop(0)[0])
                for i in range(len(gn_pend)):
                    gn_pend[i][1] -= 1
                u = pend.pop(0)
                phase2(*u)
                if u[1] == NT // 4 - 1:
                    if defer_gn:
                        gn_pend.append([u[0], 2])
                    else:
                        gn_final(u[0])

            def new_head(j):
                head_ctx[j] = (
                    wp.tile([128, T], BF16, tag="ytr", name="ytr", bufs=3),
                    wp.tile([128, 4], F32, tag="s1p", name="s1p"),
                    wp.tile([128, 4], F32, tag="s2p", name="s2p"))

            # ---- pair (0,1): bg (v + oc1 projections) fills the PE ----
            for qb in range(NT // 4):
                for j in (0, 1):
                    if qb == 0:
                        new_head(j)
                    att_rows = phase1(j, qb)
                    bg_drain(3)
                    pend.append((j, qb, att_rows))
                    if len(pend) > LAG:
                        drain_one()
            bg_drain(len(bg))

            # pair boundary: release x/weight tiles, open the partial pool
            lp_cm.__exit__(None, None, None)
            dp_cm = tc.tile_pool(name="drain", bufs=1)
            dp = dp_cm.__enter__()
            p01 = [dp.tile([128, T], BF16, tag=f"p01_{ocb}",
                           name=f"p01_{ocb}") for ocb in range(8)]

            # out-proj j=0,1 partials: fill PE during the ACT-bound
            # (2,3) pair. Emitted one ocb per (j,qb) slot.
            fill_q = list(range(8))
            _fill_rr = [0]

            def emit_fill(ocb):
                for tb in range(T // 512):
                    pt = ps_t.tile([128, 512], F32, tag="tz", name="fl")
                    for j in (0, 1):
                        nc.tensor.matmul(
                            pt[:],
                            wct[j][:, ocb * 128:(ocb + 1) * 128],
                            yt[j][:, tb * 512:(tb + 1) * 512],
                            start=(j == 0), stop=(j == 1),
                            skip_group_check=True)
                    nc.vector.tensor_copy(
                        p01[ocb][:, tb * 512:(tb + 1) * 512], pt[:])

            # ---- pair (2,3): fills + attention ----
            first23 = True
            for qb in range(NT // 4):
                for j in (2, 3):
                    if qb == 0:
                        new_head(j)
                    att_rows = phase1(j, qb)
                    pend.append((j, qb, att_rows))
                    if len(pend) > LAG:
                        drain_one()
                    if first23:
                        # fills (and head-3's ytr ring slot) need yt0/yt1:
                        # force gn(0)/gn(1) now; their pg-matmul waits hide
                        # behind the phase2(1,3) transposes just emitted
                        while gn_pend:
                            gn_final(gn_pend.pop(0)[0])
                        first23 = False
                    elif fill_q:
                        emit_fill(fill_q.pop(0))
            while pend:
                drain_one()
            while fill_q:
                emit_fill(fill_q.pop(0))
            while len(gn_pend) > 1:
                gn_final(gn_pend.pop(0)[0])

            # ================= output projection =================
            # per ocb: ident-add of the j01 partial, then j=2, then j=3.
            # gn(3) is emitted just before ocb0 so its ACT->DVE chain hides
            # behind the gn-independent ident-adds + j=2 matmuls.
            def s_halves():
                psA = ps_s.tile([128, 1024], F32, tag="s", name="psA")
                psB = ps_s.tile([128, 1024], F32, tag="s", name="psB")
                return [(psA, 0), (psA, 512), (psB, 0), (psB, 512)]

            def op_add(ocb, halves):
                for tb in range(T // 512):
                    ph, off = halves[tb]
                    mi = nc.tensor.matmul(
                        ph[:, off:off + 512],
                        ident_t[:],
                        p01[ocb][:, tb * 512:(tb + 1) * 512],
                        start=True, stop=False,
                        skip_group_check=True)
                    if tb > 0:
                        mi.ins.ldweights = False

            def op_mms23(ocb, halves):
                for j in (2, 3):
                    for tb in range(T // 512):
                        ph, off = halves[tb]
                        mi = nc.tensor.matmul(
                            ph[:, off:off + 512],
                            wct[j][:, ocb * 128:(ocb + 1) * 128],
                            yt[j][:, tb * 512:(tb + 1) * 512],
                            start=False, stop=(j == 3),
                            skip_group_check=True)
                        if tb > 0:
                            mi.ins.ldweights = False

            _fin_rr = [0]

            def op_fin(ocb, halves):
                for tb in range(T // 512):
                    ph, off = halves[tb]
                    ob = dp.tile([128, 512], BF16, tag="ob", bufs=8,
                                 name="ob")
                    r = _fin_rr[0] = (_fin_rr[0] + 1) % 2
                    if r == 0:
                        nc.vector.tensor_copy(ob[:], ph[:, off:off + 512])
                    else:
                        nc.scalar.copy(ob[:], ph[:, off:off + 512])
                    eng = (nc.sync, nc.gpsimd, nc.scalar)[tb % 3]
                    eng.dma_start(
                        outT_d[ocb * 128:(ocb + 1) * 128,
                               tb * 512:(tb + 1) * 512], ob[:])

            while gn_pend:
                gn_final(gn_pend.pop(0)[0])
            for ocb in range(8):
                halves = s_halves()
                op_add(ocb, halves)
                op_mms23(ocb, halves)
                op_fin(ocb, halves)
            dp_cm.__exit__(None, None, None)
            wp_cm.__exit__(None, None, None)

    nc.compile()
    return nc


def _prep_inputs(inputs):
    bf = ml_dtypes.bfloat16
    x = np.asarray(inputs["x"], np.float32)
    Wq1 = np.asarray(inputs["Wq1"], np.float32)
    Wq2 = np.asarray(inputs["Wq2"], np.float32)
    Wk1 = np.asarray(inputs["Wk1"], np.float32)
    Wk2 = np.asarray(inputs["Wk2"], np.float32)
    Wv = np.asarray(inputs["Wv"], np.float32)
    Wc = np.asarray(inputs["Wc"], np.float32)
    gn_w = np.asarray(inputs["gn_w"], np.float32)
    gn_b = np.asarray(inputs["gn_b"], np.float32)
    gamma = np.asarray(inputs["gamma"], np.float32)

    def sig(v):
        return 1.0 / (1.0 + np.exp(-v))

    lam = (sig(np.asarray(inputs["lq1"], np.float32).reshape(H)
               * np.asarray(inputs["lk1"], np.float32).reshape(H))
           - sig(np.asarray(inputs["lq2"], np.float32).reshape(H)
                 * np.asarray(inputs["lk2"], np.float32).reshape(H))
           + LAMBDA_INIT)

    mask = np.where(np.arange(128)[None, :] <= np.arange(128)[:, None],
                    0.0, NEG).astype(bf)
    ident = np.eye(128, dtype=np.float32).astype(bf)
    gg = (np.arange(128)[:, None] // 4 == np.arange(128)[None, :] // 4
          ).astype(np.float32)
    c1 = 1.0 - LAMBDA_INIT
    gw2 = (gn_w * gamma * c1).astype(np.float32).reshape(128, 1)
    gb2 = (gn_b * gamma * c1).astype(np.float32).reshape(128, 1)

    icon = np.zeros((128, 2), np.uint32)
    icon[:, 0] = 1
    icon[:, 1] = 0x5f375a00
    xTb = [np.ascontiguousarray(x[b].T).astype(bf) for b in range(B)]
    in_maps = []
    for core in range(N_CORES):
        b, hg = divmod(core, N_CORES // B)
        qs = hg * NH * HS          # 256-wide q/k slice
        vs = hg * NH * D           # 512-wide v / y2 slice
        lamn = np.repeat(-lam[hg * NH:(hg + 1) * NH].reshape(1, NH),
                         128, axis=0).astype(np.float32)
        def mtiles(wt, cols):
            # [1024, cols] -> merged 2-tile layout [256, 4*cols]
            return np.ascontiguousarray(
                wt.reshape(2, 4, 128, cols).transpose(0, 2, 1, 3)
                .reshape(256, 4 * cols)).astype(bf)

        in_maps.append({
            "xT": xTb[b],
            "wq1T": mtiles(Wq1[qs:qs + NH * HS, :].T, NH * HS),
            "wq2T": mtiles(Wq2[qs:qs + NH * HS, :].T, NH * HS),
            "wk1T": mtiles(Wk1[qs:qs + NH * HS, :].T, NH * HS),
            "wk2T": mtiles(Wk2[qs:qs + NH * HS, :].T, NH * HS),
            "wvT": mtiles(Wv[vs:vs + NH * D, :].T, NH * D),
            "wcT": np.ascontiguousarray(Wc[:, vs:vs + NH * D].T).astype(bf),
            "mask128": mask,
            "ident": ident,
            "gg": gg,
            "gw2": gw2,
            "gb2": gb2,
            "lamn": lamn,
            "icon": icon,
        })
    return in_maps


def kernel(**inputs):
    if "nc" not in _cache:
        _cache["nc"] = _build()
    nc = _cache["nc"]
    in_maps = _prep_inputs(inputs)
    res = bass_utils.run_bass_kernel_spmd(
        nc, in_maps, core_ids=list(range(N_CORES)),
        **_cache.get("run_kwargs", {}))
    _cache["last_result"] = res
    out = np.zeros((B, T, C), np.float32)
    for core in range(N_CORES):
        b = core // (N_CORES // B)
        out[b] += res.results[core]["outT"].T.astype(np.float32)
    return out


# revision 23
# speedup vs baseline: 1.1414x; 1.0317x over previous
"""Trainium2 Bass kernel for nn_MultiHeadDiffAttention (B=2,T=2048,C=1024,H=16).

Sharding: 8 cores = data-parallel over B(2) x tensor-parallel over 4 head-groups
(4 heads each). Each core computes q/k/v projections for its heads, causal
differential attention, per-head GroupNorm, and a partial output projection
(its 512 columns of y2 against Wc). Host sums the 4 partials per batch.

v3 schedule (over the v2 layout):
  - x DMA'd first across all 4 issue queues; q1/k1 then q2/k2 weights next,
    so attention S-matmuls start ~30us in instead of ~60us.
  - only the oc=0 (heads 0-1) q/k projections run up front; oc=1 and the v
    projection drain as background PE work inside the head-0/1 loops.
  - head pair (2,3) has no projection work left to fill PE gaps (its phase
    is ACT-exp-bound), so the j=0/1 half of the output projection runs
    there instead: partial (y0*Wc0 + y1*Wc1) per ocb into bf16 SBUF tiles
    (space freed by closing the loads pool at the pair boundary), added
    back in the final phase via an identity-stationary matmul.
  - combine att' = e1 + s*e2 split into tensor_scalar (4x DVE mode) +
    tensor_tensor (2x) instead of one 1x scalar_tensor_tensor pass.
  - psum->sbuf evictions rotate across DVE/ACT/GPSIMD so neither ACT (exp)
    nor DVE (combine/casts) eats them all.
  - outT is DMA'd in bf16 (halves the output traffic); host upcasts.
"""

import sys

for _p in ("/opt/trn_rl_repo", "/root/.axon_site/_ro/trn_rl_repo"):
    if _p not in sys.path:
        sys.path.insert(0, _p)

import math
import numpy as np
import ml_dtypes

import concourse.bass as bass
import concourse.bacc as bacc
import concourse.tile as tile
import concourse.mybir as mybir
from concourse import bass_utils

F32 = mybir.dt.float32
BF16 = mybir.dt.bfloat16
AF = mybir.ActivationFunctionType
ALU = mybir.AluOpType

B, T, C = 2, 2048, 1024
H = 16
HS = C // H           # 64
D = 2 * HS            # 128 v-channels per head
NH = 4                # heads per core
N_CORES = 8
NT = T // 128         # 16 q-tiles
LAMBDA_INIT = 0.8 - 0.6 * math.exp(-0.3 * (12 - 1))
EPS = 1e-5
SCALE = 1.0 / math.sqrt(HS)
NEG = -30000.0

_cache = {}


def _build(T=T, trace_sim=False, nh=NH):
    NT = T // 128
    nc = bacc.Bacc("TRN2", target_bir_lowering=False, debug=False,
                   num_devices=N_CORES)

    def din(name, shape, dt=BF16):
        return nc.dram_tensor(name, shape, dt, kind="ExternalInput").ap()

    xT_d = din("xT", [C, T])
    # q/k/v weights arrive in host-merged tile layout: [2 tiles x 128
    # partitions, 4 chunks x cols] so each needs only a few DMAs
    wq1_d = din("wq1T", [256, 4 * NH * HS])
    wq2_d = din("wq2T", [256, 4 * NH * HS])
    wk1_d = din("wk1T", [256, 4 * NH * HS])
    wk2_d = din("wk2T", [256, 4 * NH * HS])
    wv_d = din("wvT", [256, 4 * NH * D])
    wc_d = din("wcT", [NH * D, C])
    mask_d = din("mask128", [128, 128])
    ident_d = din("ident", [128, 128])
    gg_d = din("gg", [128, 128], F32)
    gw2_d = din("gw2", [128, 1], F32)
    gb2_d = din("gb2", [128, 1], F32)
    lamn_d = din("lamn", [128, NH], F32)
    icon_d = din("icon", [128, 2], mybir.dt.uint32)
    outT_d = nc.dram_tensor("outT", [C, T], BF16, kind="ExternalOutput").ap()

    with tile.TileContext(nc, trace_sim=trace_sim) as tc:
        with tc.tile_pool(name="persist", bufs=1) as pp, \
             tc.tile_pool(name="ps_s", bufs=3, space="PSUM") as ps_s, \
             tc.tile_pool(name="ps_t", bufs=2, space="PSUM") as ps_t:

            # ---- PE warmup: ramp the tensor-engine P-state during the
            # input DMA window with dummy back-to-back matmuls ----
            wa_t = pp.tile([128, 128], BF16, tag="wa")
            nc.vector.memset(wa_t[:], 0.0)
            for _ in range(64):
                psw = ps_s.tile([128, 1024], F32, tag="s")
                nc.tensor.matmul(psw[:, 0:128], wa_t[:], wa_t[:],
                                 start=True, stop=True)

            # ---- persistent small tiles ----
            mask_t = pp.tile([128, 128], BF16, tag="mask")
            ident_t = pp.tile([128, 128], BF16, tag="ident")
            gg_t = pp.tile([128, 128], F32, tag="gg")
            gw2_t = pp.tile([128, 1], F32, tag="gw2")
            gb2_t = pp.tile([128, 1], F32, tag="gb2")
            lamn_t = pp.tile([128, NH], F32, tag="lamn")
            c15_t = pp.tile([128, 1], F32, tag="c15")
            nc.vector.memset(c15_t[:], 1.5)
            icon_t = pp.tile([128, 2], mybir.dt.uint32, tag="icon")

            # ---- persistent activation tensors ----
            q1t = [pp.tile([128, T], BF16, tag=f"q1t{i}", name=f"q1t{i}") for i in range(2)]
            q2t = [pp.tile([128, T], BF16, tag=f"q2t{i}", name=f"q2t{i}") for i in range(2)]
            k1t = [pp.tile([128, T], BF16, tag=f"k1t{i}", name=f"k1t{i}") for i in range(2)]
            k2t = [pp.tile([128, T], BF16, tag=f"k2t{i}", name=f"k2t{i}") for i in range(2)]
            vt = [pp.tile([128, NH * D], BF16, tag=f"vt{i}", name=f"vt{i}") for i in range(NT)]
            # yT per head [D=128, T] bf16 (post-groupnorm)
            yt = [pp.tile([128, T], BF16, tag=f"yt{j}", name=f"yt{j}") for j in range(NH)]
            # wcT: [512, C] as 4 f-chunk tiles (one per head)
            wct = [pp.tile([128, C], BF16, tag=f"wct{j}", name=f"wct{j}") for j in range(NH)]

            # attention working pool opened BEFORE the loads pool so the
            # loads pool can be released first (pools close LIFO)
            wp_cm = tc.tile_pool(name="aw", bufs=2)
            wp = wp_cm.__enter__()

            # ================= q/k projection loads =================
            lp_cm = tc.tile_pool(name="loads", bufs=1)
            lp = lp_cm.__enter__()
            xt = [lp.tile([128, T], BF16, tag=f"xt{i}", name=f"xt{i}") for i in range(8)]
            wqm = {}
            for nm in ("q1", "q2", "k1", "k2"):
                wqm[nm] = [lp.tile([128, 4 * NH * HS], BF16,
                                   tag=f"w{nm}{h}", name=f"w{nm}{h}")
                           for h in range(2)]
            wvm = [lp.tile([128, 4 * NH * D], BF16, tag=f"wvm{h}",
                           name=f"wvm{h}") for h in range(2)]

            def wq_sl(nm, cc, oc):
                return wqm[nm][cc // 4][:, (cc % 4) * 256 + oc * 128:
                                        (cc % 4) * 256 + oc * 128 + 128]

            def wv_sl(cc):
                return wvm[cc // 4][:, (cc % 4) * 512:(cc % 4) * 512 + 512]

            # ---- DMA schedule (3 issue queues: SP/ACT/Pool): x first,
            # then q1/k1 weights, then q2/k2, then wv/wc + small tiles ----
            queues = [nc.sync, nc.scalar, nc.gpsimd]
            for i in range(8):
                queues[i % 3].dma_start(xt[i][:], xT_d[i * 128:(i + 1) * 128, :])
            # small tiles needed early in attention (tiny; on the queue
            # with one fewer x chunk)
            nc.gpsimd.dma_start(mask_t[:], mask_d)
            nc.gpsimd.dma_start(ident_t[:], ident_d)
            nc.gpsimd.dma_start(lamn_t[:], lamn_d)
            for nm, d_ap, eng in (("q1", wq1_d, nc.sync),
                                  ("k1", wk1_d, nc.scalar),
                                  ("q2", wq2_d, nc.gpsimd),
                                  ("k2", wk2_d, nc.gpsimd)):
                for h in range(2):
                    for cf in range(2):
                        eng.dma_start(
                            wqm[nm][h][:, cf * 512:(cf + 1) * 512],
                            d_ap[h * 128:(h + 1) * 128,
                                 cf * 512:(cf + 1) * 512])
            # wv needed by the first background v chunks (~35us in)
            for h in range(2):
                for cf in range(2):
                    eng = nc.sync if cf == 0 else nc.scalar
                    eng.dma_start(
                        wvm[h][:, cf * 1024:(cf + 1) * 1024],
                        wv_d[h * 128:(h + 1) * 128,
                             cf * 1024:(cf + 1) * 1024])
            # wc + groupnorm consts needed late
            for j in range(NH):
                eng = (nc.sync, nc.scalar)[j % 2]
                eng.dma_start(wct[j][:], wc_d[j * 128:(j + 1) * 128, :])
            nc.gpsimd.dma_start(gg_t[:], gg_d)
            nc.gpsimd.dma_start(gw2_t[:], gw2_d)
            nc.gpsimd.dma_start(gb2_t[:], gb2_d)
            nc.gpsimd.dma_start(icon_t[:], icon_d)

            # qT/kT projections: out [o=128, t=512] = W^T_chunk.T @ xT
            _qk_ps = {}

            def emit_qk(nm, dst, oc, half):
                if half == 0:
                    _qk_ps[(nm, oc)] = (ps_s.tile([128, 1024], F32, tag="s", name="qkA"),
                                        ps_s.tile([128, 1024], F32, tag="s", name="qkB"))
                psA, psB = _qk_ps[(nm, oc)]
                for cc in range(4 * half, 4 * half + 4):
                    for tb in range(T // 512):
                        ph = (psA, psB)[tb // 2]
                        mi = nc.tensor.matmul(
                            ph[:, (tb % 2) * 512:(tb % 2) * 512 + 512],
                            wq_sl(nm, cc, oc),
                            xt[cc][:, tb * 512:(tb + 1) * 512],
                            start=(cc == 0), stop=(cc == 7),
                            skip_group_check=True)
                        if tb > 0:
                            mi.ins.ldweights = False
                if half == 1:
                    for tb in range(T // 512):
                        ph = (psA, psB)[tb // 2]
                        src = ph[:, (tb % 2) * 512:(tb % 2) * 512 + 512]
                        if nm in ("q1", "k1"):
                            nc.scalar.copy(
                                dst[oc][:, tb * 512:(tb + 1) * 512], src)
                        else:
                            nc.vector.tensor_copy(
                                dst[oc][:, tb * 512:(tb + 1) * 512], src)

            def emit_qk_full(nm, dst, oc):
                emit_qk(nm, dst, oc, 0)
                emit_qk(nm, dst, oc, 1)

            # oc=0 projections up front (heads 0-1 attention needs them)
            for nm, dst in (("q1", q1t), ("k1", k1t), ("q2", q2t), ("k2", k2t)):
                emit_qk_full(nm, dst, 0)

            # v projection chunks are emitted lazily
            v_done = [False] * NT

            def emit_v(tch):
                if v_done[tch]:
                    return
                v_done[tch] = True
                ps = ps_t.tile([128, NH * D], F32, tag="tz")
                for cc in range(8):
                    nc.tensor.matmul(
                        ps[:],
                        xt[cc][:, tch * 128:(tch + 1) * 128],
                        wv_sl(cc),
                        start=(cc == 0), stop=(cc == 7))
                if tch % 2 == 0:
                    nc.vector.tensor_copy(vt[tch][:], ps[:])
                else:
                    nc.scalar.copy(vt[tch][:], ps[:])

            # background PE work drained between attention phases of
            # heads 0-1: v chunks (just-in-time via the phase2 safety),
            # then the oc=1 projections (needed only by pair (2,3))
            bg = [(lambda t=t: emit_v(t)) for t in range(NT)]
            for nm, dst in (("q1", q1t), ("k1", k1t),
                            ("q2", q2t), ("k2", k2t)):
                bg.append(lambda nm=nm, dst=dst: emit_qk_full(nm, dst, 1))

            def bg_drain(n):
                for _ in range(min(n, len(bg))):
                    bg.pop(0)()

            # ================= attention per head =================
            head_ctx = {}

            def phase1_qt(j, qb, qq, AB):
                oc, po = divmod(j * HS, 128)
                qt = qb * 4 + qq
                nk = qt + 1
                nkb2 = (nk + 7) // 8   # 1024-wide S psum tiles
                e1 = wp.tile([128, T], BF16, tag="e1", name="e1", bufs=4)
                e2 = wp.tile([128, T], BF16, tag="e2", name="e2", bufs=3)
                dd = wp.tile([128, 2], F32, tag="dd", name="dd", bufs=4)
                d1c = wp.tile([128, 2], F32, tag="d1c", name="d1c", bufs=4)
                d2c = wp.tile([128, 2], F32, tag="d2c", name="d2c", bufs=4)
                for mi, (qsrc, ksrc, erow, dcol) in enumerate(
                        ((q1t, k1t, e1, d1c), (q2t, k2t, e2, d2c))):
                    for kb in range(nkb2):
                        w = min(1024, nk * 128 - kb * 1024)
                        ps = ps_s.tile([128, 1024], F32, tag="s",
                                       name="ps")
                        off = qt * 128 - kb * 1024  # diag block col
                        for hf in range(2):
                            wh = min(512, w - hf * 512)
                            if wh <= 0:
                                break
                            diag_here = (kb == nkb2 - 1 and
                                         hf * 512 <= off < hf * 512 + wh)
                            mm = nc.tensor.matmul(
                                ps[:, hf * 512:hf * 512 + wh],
                                qsrc[oc][po:po + HS,
                                         qt * 128:(qt + 1) * 128],
                                ksrc[oc][po:po + HS,
                                         kb * 1024 + hf * 512:
                                         kb * 1024 + hf * 512 + wh],
                                start=True, stop=not diag_here,
                                skip_group_check=diag_here)
                            if kb + hf > 0:
                                mm.ins.ldweights = False
                        if kb == nkb2 - 1:
                            # mask diagonal 128-block on PE
                            nc.tensor.matmul(
                                ps[:, off:off + 128],
                                ident_t[:], mask_t[:],
                                start=False, stop=True,
                                skip_group_check=True)
                        # accum straight into dd when a single psum covers
                        # the row (saves two DVE copies per qt)
                        acc = (dd[:, mi:mi + 1] if nkb2 == 1
                               else dcol[:, kb:kb + 1])
                        nc.scalar.activation(
                            erow[:, kb * 1024:kb * 1024 + w],
                            ps[:, :w], AF.Exp, scale=SCALE,
                            accum_out=acc)
                # denominators -> rr = [1/D1, 1/D2]
                rr = wp.tile([128, 2], F32, tag="rr", name="rr", bufs=4)
                if nkb2 > 1:
                    nc.vector.tensor_reduce(
                        dd[:, 0:1], d1c[:, 0:nkb2],
                        axis=mybir.AxisListType.X, op=ALU.add)
                    nc.vector.tensor_reduce(
                        dd[:, 1:2], d2c[:, 0:nkb2],
                        axis=mybir.AxisListType.X, op=ALU.add)
                nc.vector.reciprocal(rr[:], dd[:, 0:2])
                # sc2 = -lam / D2 (per-partition scalar)
                sc2 = wp.tile([128, 1], F32, tag="sc2", name="sc2", bufs=4)
                nc.vector.tensor_tensor(sc2[:], rr[:, 1:2],
                                        lamn_t[:, j:j + 1], ALU.mult)
                # att = e1*r1 + e2*sc2 (fully normalized diff-attention
                # row): tensor_scalar (4x) + scalar_tensor_tensor
                etmp = wp.tile([128, T], BF16, tag="etmp", name="etmp",
                               bufs=1)
                nc.vector.tensor_scalar(
                    etmp[:, :nk * 128], e2[:, :nk * 128], sc2[:, 0:1],
                    None, op0=ALU.mult)
                nc.vector.scalar_tensor_tensor(
                    e1[:, :nk * 128], e1[:, :nk * 128], rr[:, 0:1],
                    etmp[:, :nk * 128], op0=ALU.mult, op1=ALU.add)
                # transpose this attention row on the DMA xbar:
                # [q=128, nk*128] -> nk transposed blocks [k=128, 128]
                nc.sync.dma_start_transpose(
                    AB[:, 0:nk, qq, :], e1[:, :nk * 128])

            def phase1_pair(jA, jB, qb):
                # qt-interleaved emission across the head pair: the PE always
                # has the other head's independent S-chunk while ACT/DVE
                # drain this one's exp/combine chain
                ABs = {}
                for j in (jA, jB):
                    # AB layout [k=128, kc, qq, q-col]: z-matmul rhs slices
                    # contiguous (strided moving operands are silently wrong
                    # on HW; strided DMA-transpose OUT is fine)
                    ABs[j] = wp.tile([128, NT, 4, 128], BF16, tag="AB",
                                     name="AB", bufs=2)
                for qq in range(4):
                    for j in (jA, jB):
                        phase1_qt(j, qb, qq, ABs[j])
                return ABs

            def phase2(j, qb, AB):
                ytr, s1p, s2p = head_ctx[j]
                nkc = qb * 4 + 4
                # yT[d, qblk] = sum_kc v_kc.T @ attT_kc   (N=512)
                py = ps_t.tile([128, 512], F32, tag="tz", name="py")
                for kc in range(nkc):
                    emit_v(kc)
                    qq0 = max(0, kc - qb * 4)
                    zw = qq0 * 128
                    nc.tensor.matmul(
                        py[:, zw:],
                        vt[kc][:, j * 128:(j + 1) * 128],
                        AB[:, kc, qq0:4, :].rearrange("p q c -> p (q c)"),
                        start=(kc == 0), stop=(kc == nkc - 1),
                        skip_group_check=True)
                # copy to ytr with fused stats accumulation (ACT)
                nc.scalar.activation(
                    ytr[:, qb * 512:(qb + 1) * 512], py[:], AF.Copy,
                    accum_out=s1p[:, qb:qb + 1])
                ysq = wp.tile([128, 512], BF16, tag="ysq", name="ysq",
                              bufs=1)
                nc.scalar.activation(
                    ysq[:], py[:], AF.Square,
                    accum_out=s2p[:, qb:qb + 1])

            def gn_final(j):
                ytr, s1p, s2p = head_ctx[j]
                s12 = wp.tile([128, 2], F32, tag="s12", name="s12")
                nc.vector.tensor_reduce(s12[:, 0:1], s1p[:, 0:NT // 4],
                                        axis=mybir.AxisListType.X, op=ALU.add)
                nc.vector.tensor_reduce(s12[:, 1:2], s2p[:, 0:NT // 4],
                                        axis=mybir.AxisListType.X, op=ALU.add)
                pg = ps_t.tile([128, 2], F32, tag="tz", name="pg")
                nc.tensor.matmul(pg[:], gg_t[:], s12[:], start=True, stop=True)
                # mneg = -mean; nvar = mean^2 - E[y^2] = -var
                mneg = wp.tile([128, 1], F32, tag="mneg", name="mneg")
                nc.scalar.mul(mneg[:], pg[:, 0:1], -1.0 / (T * 4))
                msq = wp.tile([128, 1], F32, tag="msq")
                nc.scalar.mul(msq[:], pg[:, 1:2], 1.0 / (T * 4))
                nvar = wp.tile([128, 1], F32, tag="nvar")
                nc.vector.scalar_tensor_tensor(
                    nvar[:], mneg[:], mneg[:, 0:1], msq[:],
                    op0=ALU.mult, op1=ALU.subtract)
                vpe = wp.tile([128, 1], F32, tag="vpe")
                nc.vector.tensor_scalar(vpe[:], nvar[:], -1.0, EPS,
                                        op0=ALU.mult, op1=ALU.add)  # var+eps
                # rsqrt(var+eps) on DVE only: quake seed + Newton iters
                rstd = wp.tile([128, 1], F32, tag="rstd")
                yi = wp.tile([128, 1], F32, tag="yi")
                nc.vector.tensor_tensor(yi.bitcast(mybir.dt.uint32)[:],
                                        vpe.bitcast(mybir.dt.uint32)[:],
                                        icon_t[:, 0:1],
                                        ALU.logical_shift_right)
                nc.vector.tensor_tensor(yi.bitcast(mybir.dt.uint32)[:],
                                        icon_t[:, 1:2],
                                        yi.bitcast(mybir.dt.uint32)[:],
                                        ALU.subtract)
                vneg = wp.tile([128, 1], F32, tag="vneg")
                nc.vector.tensor_scalar_mul(vneg[:], vpe[:], -0.5)
                ytmp = wp.tile([128, 1], F32, tag="ytmp")
                for _ in range(2):
                    nc.vector.tensor_tensor(ytmp[:], yi[:], yi[:], ALU.mult)
                    nc.vector.scalar_tensor_tensor(
                        ytmp[:], ytmp[:], vneg[:, 0:1], c15_t[:],
                        op0=ALU.mult, op1=ALU.add)  # 1.5 - 0.5 v y^2
                    nc.vector.tensor_tensor(yi[:], yi[:], ytmp[:], ALU.mult)
                nc.vector.tensor_copy(rstd[:], yi[:])
                aff_a = wp.tile([128, 1], F32, tag="aff_a")
                nc.vector.tensor_tensor(aff_a[:], rstd[:], gw2_t[:], ALU.mult)
                aff_b = wp.tile([128, 1], F32, tag="aff_b")
                nc.vector.scalar_tensor_tensor(
                    aff_b[:], mneg[:], aff_a[:, 0:1], gb2_t[:],
                    op0=ALU.mult, op1=ALU.add)  # gb2 - mean*aff_a
                # affine on DVE (4x): yt = ytr*aff_a + aff_b
                nc.vector.tensor_scalar(yt[j][:], ytr[:], aff_a[:, 0:1],
                                        aff_b[:, 0:1],
                                        op0=ALU.mult, op1=ALU.add)

            def new_head(j):
                head_ctx[j] = (
                    wp.tile([128, T], BF16, tag="ytr", name="ytr", bufs=3),
                    wp.tile([128, 4], F32, tag="s1p", name="s1p"),
                    wp.tile([128, 4], F32, tag="s2p", name="s2p"))

            # qb order (1,2,3,0): the serial end-of-pair chain (last exp ->
            # combine -> transpose -> z -> stats -> gn) runs on the smallest
            # q-block, shrinking the pair-boundary latency
            QBS = (1, 2, 3, 0)

            # ---- pair (0,1): bg (v + oc1 projections) fills the PE ----
            for j in (0, 1):
                new_head(j)
            for qb in QBS:
                ABs = phase1_pair(0, 1, qb)
                bg_drain(5)
                phase2(0, qb, ABs[0])
                phase2(1, qb, ABs[1])
            bg_drain(len(bg))
            gn_final(0)
            gn_final(1)

            # pair boundary: release x/weight tiles, open the partial pool
            lp_cm.__exit__(None, None, None)
            dp_cm = tc.tile_pool(name="drain", bufs=1)
            dp = dp_cm.__enter__()
            p01 = [dp.tile([128, T], BF16, tag=f"p01_{ocb}",
                           name=f"p01_{ocb}") for ocb in range(8)]

            # out-proj j=0,1 partials: fill PE during the ACT-bound
            # (2,3) pair. Two ocbs per qb slot.
            fill_q = list(range(8))

            def emit_fill(ocb):
                for tb in range(T // 512):
                    pt = ps_t.tile([128, 512], F32, tag="tz", name="fl")
                    for j in (0, 1):
                        nc.tensor.matmul(
                            pt[:],
                            wct[j][:, ocb * 128:(ocb + 1) * 128],
                            yt[j][:, tb * 512:(tb + 1) * 512],
                            start=(j == 0), stop=(j == 1),
                            skip_group_check=True)
                    nc.vector.tensor_copy(
                        p01[ocb][:, tb * 512:(tb + 1) * 512], pt[:])

            # ---- pair (2,3): fills + attention ----
            for j in (2, 3):
                new_head(j)
            for qb in QBS:
                ABs = phase1_pair(2, 3, qb)
                for _ in range(2):
                    if fill_q:
                        emit_fill(fill_q.pop(0))
                phase2(2, qb, ABs[2])
                phase2(3, qb, ABs[3])
            while fill_q:
                emit_fill(fill_q.pop(0))
            gn_final(2)
            gn_final(3)

            # ================= output projection =================
            # per ocb: ident-add of the j01 partial, then j=2, then j=3.
            # gn(3) is emitted just before ocb0 so its ACT->DVE chain hides
            # behind the gn-independent ident-adds + j=2 matmuls.
            def s_halves():
                psA = ps_s.tile([128, 1024], F32, tag="s", name="psA")
                psB = ps_s.tile([128, 1024], F32, tag="s", name="psB")
                return [(psA, 0), (psA, 512), (psB, 0), (psB, 512)]

            def op_add(ocb, halves):
                for tb in range(T // 512):
                    ph, off = halves[tb]
                    mi = nc.tensor.matmul(
                        ph[:, off:off + 512],
                        ident_t[:],
                        p01[ocb][:, tb * 512:(tb + 1) * 512],
                        start=True, stop=False,
                        skip_group_check=True)
                    if tb > 0:
                        mi.ins.ldweights = False

            def op_mms23(ocb, halves):
                for j in (2, 3):
                    for tb in range(T // 512):
                        ph, off = halves[tb]
                        mi = nc.tensor.matmul(
                            ph[:, off:off + 512],
                            wct[j][:, ocb * 128:(ocb + 1) * 128],
                            yt[j][:, tb * 512:(tb + 1) * 512],
                            start=False, stop=(j == 3),
                            skip_group_check=True)
                        if tb > 0:
                            mi.ins.ldweights = False

            _fin_rr = [0]

            def op_fin(ocb, halves):
                for tb in range(T // 512):
                    ph, off = halves[tb]
                    ob = dp.tile([128, 512], BF16, tag="ob", bufs=8,
                                 name="ob")
                    r = _fin_rr[0] = (_fin_rr[0] + 1) % 2
                    if r == 0:
                        nc.vector.tensor_copy(ob[:], ph[:, off:off + 512])
                    else:
                        nc.scalar.copy(ob[:], ph[:, off:off + 512])
                    eng = (nc.sync, nc.gpsimd, nc.scalar)[tb % 3]
                    eng.dma_start(
                        outT_d[ocb * 128:(ocb + 1) * 128,
                               tb * 512:(tb + 1) * 512], ob[:])

            for ocb in range(8):
                halves = s_halves()
                op_add(ocb, halves)
                op_mms23(ocb, halves)
                op_fin(ocb, halves)
            dp_cm.__exit__(None, None, None)
            wp_cm.__exit__(None, None, None)

    nc.compile()
    return nc


def _prep_inputs(inputs):
    bf = ml_dtypes.bfloat16
    x = np.asarray(inputs["x"], np.float32)
    Wq1 = np.asarray(inputs["Wq1"], np.float32)
    Wq2 = np.asarray(inputs["Wq2"], np.float32)
    Wk1 = np.asarray(inputs["Wk1"], np.float32)
    Wk2 = np.asarray(inputs["Wk2"], np.float32)
    Wv = np.asarray(inputs["Wv"], np.float32)
    Wc = np.asarray(inputs["Wc"], np.float32)
    gn_w = np.asarray(inputs["gn_w"], np.float32)
    gn_b = np.asarray(inputs["gn_b"], np.float32)
    gamma = np.asarray(inputs["gamma"], np.float32)

    def sig(v):
        return 1.0 / (1.0 + np.exp(-v))

    lam = (sig(np.asarray(inputs["lq1"], np.float32).reshape(H)
               * np.asarray(inputs["lk1"], np.float32).reshape(H))
           - sig(np.asarray(inputs["lq2"], np.float32).reshape(H)
                 * np.asarray(inputs["lk2"], np.float32).reshape(H))
           + LAMBDA_INIT)

    mask = np.where(np.arange(128)[None, :] <= np.arange(128)[:, None],
                    0.0, NEG).astype(bf)
    ident = np.eye(128, dtype=np.float32).astype(bf)
    gg = (np.arange(128)[:, None] // 4 == np.arange(128)[None, :] // 4
          ).astype(np.float32)
    c1 = 1.0 - LAMBDA_INIT
    gw2 = (gn_w * gamma * c1).astype(np.float32).reshape(128, 1)
    gb2 = (gn_b * gamma * c1).astype(np.float32).reshape(128, 1)

    icon = np.zeros((128, 2), np.uint32)
    icon[:, 0] = 1
    icon[:, 1] = 0x5f375a00
    xTb = [np.ascontiguousarray(x[b].T).astype(bf) for b in range(B)]
    in_maps = []
    for core in range(N_CORES):
        b, hg = divmod(core, N_CORES // B)
        qs = hg * NH * HS          # 256-wide q/k slice
        vs = hg * NH * D           # 512-wide v / y2 slice
        lamn = np.repeat(-lam[hg * NH:(hg + 1) * NH].reshape(1, NH),
                         128, axis=0).astype(np.float32)
        def mtiles(wt, cols):
            # [1024, cols] -> merged 2-tile layout [256, 4*cols]
            return np.ascontiguousarray(
                wt.reshape(2, 4, 128, cols).transpose(0, 2, 1, 3)
                .reshape(256, 4 * cols)).astype(bf)

        in_maps.append({
            "xT": xTb[b],
            "wq1T": mtiles(Wq1[qs:qs + NH * HS, :].T, NH * HS),
            "wq2T": mtiles(Wq2[qs:qs + NH * HS, :].T, NH * HS),
            "wk1T": mtiles(Wk1[qs:qs + NH * HS, :].T, NH * HS),
            "wk2T": mtiles(Wk2[qs:qs + NH * HS, :].T, NH * HS),
            "wvT": mtiles(Wv[vs:vs + NH * D, :].T, NH * D),
            "wcT": np.ascontiguousarray(Wc[:, vs:vs + NH * D].T).astype(bf),
            "mask128": mask,
            "ident": ident,
            "gg": gg,
            "gw2": gw2,
            "gb2": gb2,
            "lamn": lamn,
            "icon": icon,
        })
    return in_maps


def kernel(**inputs):
    if "nc" not in _cache:
        _cache["nc"] = _build()
    nc = _cache["nc"]
    in_maps = _prep_inputs(inputs)
    res = bass_utils.run_bass_kernel_spmd(
        nc, in_maps, core_ids=list(range(N_CORES)),
        **_cache.get("run_kwargs", {}))
    _cache["last_result"] = res
    out = np.zeros((B, T, C), np.float32)
    for core in range(N_CORES):
        b = core // (N_CORES // B)
        out[b] += res.results[core]["outT"].T.astype(np.float32)
    return out


# revision 36
# speedup vs baseline: 1.1940x; 1.0461x over previous
"""Trainium2 Bass kernel for nn_MultiHeadDiffAttention (B=2,T=2048,C=1024,H=16).

Sharding: 8 cores = data-parallel over B(2) x tensor-parallel over 4 head-groups
(4 heads each). Each core computes q/k/v projections for its heads, causal
differential attention, per-head GroupNorm, and a partial output projection
(its 512 columns of y2 against Wc). Host sums the 4 partials per batch.

v3 schedule (over the v2 layout):
  - x DMA'd first across all 4 issue queues; q1/k1 then q2/k2 weights next,
    so attention S-matmuls start ~30us in instead of ~60us.
  - only the oc=0 (heads 0-1) q/k projections run up front; oc=1 and the v
    projection drain as background PE work inside the head-0/1 loops.
  - head pair (2,3) has no projection work left to fill PE gaps (its phase
    is ACT-exp-bound), so the j=0/1 half of the output projection runs
    there instead: partial (y0*Wc0 + y1*Wc1) per ocb into bf16 SBUF tiles
    (space freed by closing the loads pool at the pair boundary), added
    back in the final phase via an identity-stationary matmul.
  - combine att' = e1 + s*e2 split into tensor_scalar (4x DVE mode) +
    tensor_tensor (2x) instead of one 1x scalar_tensor_tensor pass.
  - psum->sbuf evictions rotate across DVE/ACT/GPSIMD so neither ACT (exp)
    nor DVE (combine/casts) eats them all.
  - outT is DMA'd in bf16 (halves the output traffic); host upcasts.
"""

import sys

for _p in ("/opt/trn_rl_repo", "/root/.axon_site/_ro/trn_rl_repo"):
    if _p not in sys.path:
        sys.path.insert(0, _p)

import math
import numpy as np
import ml_dtypes

import concourse.bass as bass
import concourse.bacc as bacc
import concourse.tile as tile
import concourse.mybir as mybir
from concourse import bass_utils

F32 = mybir.dt.float32
BF16 = mybir.dt.bfloat16
AF = mybir.ActivationFunctionType
ALU = mybir.AluOpType

B, T, C = 2, 2048, 1024
H = 16
HS = C // H           # 64
D = 2 * HS            # 128 v-channels per head
NH = 4                # heads per core
N_CORES = 8
NT = T // 128         # 16 q-tiles
LAMBDA_INIT = 0.8 - 0.6 * math.exp(-0.3 * (12 - 1))
EPS = 1e-5
SCALE = 1.0 / math.sqrt(HS)
NEG = -30000.0

_cache = {}


def _build(T=T, trace_sim=False, nh=NH):
    NT = T // 128
    nc = bacc.Bacc("TRN2", target_bir_lowering=False, debug=False,
                   num_devices=N_CORES)

    def din(name, shape, dt=BF16):
        return nc.dram_tensor(name, shape, dt, kind="ExternalInput").ap()

    xT_d = din("xT", [C, T])
    # q/k/v weights arrive in host-merged tile layout: [2 tiles x 128
    # partitions, 4 chunks x cols] so each needs only a few DMAs
    wq1_d = din("wq1T", [256, 4 * NH * HS])
    wq2_d = din("wq2T", [256, 4 * NH * HS])
    wk1_d = din("wk1T", [256, 4 * NH * HS])
    wk2_d = din("wk2T", [256, 4 * NH * HS])
    wv_d = din("wvT", [256, 4 * NH * D])
    wc_d = din("wcT", [NH * D, C])
    mask_d = din("mask128", [128, 128])
    ident_d = din("ident", [128, 128])
    gg_d = din("gg", [128, 128], F32)
    gw2_d = din("gw2", [128, 1], F32)
    gb2_d = din("gb2", [128, 1], F32)
    lamn_d = din("lamn", [128, NH], F32)
    icon_d = din("icon", [128, 2], mybir.dt.uint32)
    outT_d = nc.dram_tensor("outT", [C, T], BF16, kind="ExternalOutput").ap()

    with tile.TileContext(nc, trace_sim=trace_sim) as tc:
        with tc.tile_pool(name="persist", bufs=1) as pp, \
             tc.tile_pool(name="ps_s", bufs=3, space="PSUM") as ps_s, \
             tc.tile_pool(name="ps_t", bufs=2, space="PSUM") as ps_t:

            # ---- PE warmup: ramp the tensor-engine P-state during the
            # input DMA window with dummy back-to-back matmuls ----
            wa_t = pp.tile([128, 128], BF16, tag="wa")
            nc.vector.memset(wa_t[:], 0.0)
            for _ in range(64):
                psw = ps_s.tile([128, 1024], F32, tag="s")
                nc.tensor.matmul(psw[:, 0:128], wa_t[:], wa_t[:],
                                 start=True, stop=True)

            # ---- persistent small tiles ----
            mask_t = pp.tile([128, 128], BF16, tag="mask")
            ident_t = pp.tile([128, 128], BF16, tag="ident")
            gg_t = pp.tile([128, 128], F32, tag="gg")
            gw2_t = pp.tile([128, 1], F32, tag="gw2")
            gb2_t = pp.tile([128, 1], F32, tag="gb2")
            lamn_t = pp.tile([128, NH], F32, tag="lamn")
            c15_t = pp.tile([128, 1], F32, tag="c15")
            nc.vector.memset(c15_t[:], 1.5)
            icon_t = pp.tile([128, 2], mybir.dt.uint32, tag="icon")

            # ---- persistent activation tensors ----
            q1t = [pp.tile([128, T], BF16, tag=f"q1t{i}", name=f"q1t{i}") for i in range(2)]
            q2t = [pp.tile([128, T], BF16, tag=f"q2t{i}", name=f"q2t{i}") for i in range(2)]
            k1t = [pp.tile([128, T], BF16, tag=f"k1t{i}", name=f"k1t{i}") for i in range(2)]
            k2t = [pp.tile([128, T], BF16, tag=f"k2t{i}", name=f"k2t{i}") for i in range(2)]
            vt = [pp.tile([128, NH * D], BF16, tag=f"vt{i}", name=f"vt{i}") for i in range(NT)]
            # yT per head [D=128, T] bf16 (post-groupnorm)
            yt = [pp.tile([128, T], BF16, tag=f"yt{j}", name=f"yt{j}") for j in range(NH)]
            # wcT: [512, C] as 4 f-chunk tiles (one per head)
            wct = [pp.tile([128, C], BF16, tag=f"wct{j}", name=f"wct{j}") for j in range(NH)]

            # attention working pool opened BEFORE the loads pool so the
            # loads pool can be released first (pools close LIFO)
            wp_cm = tc.tile_pool(name="aw", bufs=2)
            wp = wp_cm.__enter__()

            # ================= q/k projection loads =================
            lp_cm = tc.tile_pool(name="loads", bufs=1)
            lp = lp_cm.__enter__()
            xt = [lp.tile([128, T], BF16, tag=f"xt{i}", name=f"xt{i}") for i in range(8)]
            wqm = {}
            for nm in ("q1", "q2", "k1", "k2"):
                wqm[nm] = [lp.tile([128, 4 * NH * HS], BF16,
                                   tag=f"w{nm}{h}", name=f"w{nm}{h}")
                           for h in range(2)]
            wvm = [lp.tile([128, 4 * NH * D], BF16, tag=f"wvm{h}",
                           name=f"wvm{h}") for h in range(2)]

            def wq_sl(nm, cc, oc):
                return wqm[nm][cc // 4][:, (cc % 4) * 256 + oc * 128:
                                        (cc % 4) * 256 + oc * 128 + 128]

            def wv_sl(cc):
                return wvm[cc // 4][:, (cc % 4) * 512:(cc % 4) * 512 + 512]

            # ---- DMA schedule (3 issue queues: SP/ACT/Pool): x chunks 0-3
            # and the h=0 halves of q1/k1/q2/k2 first (the first 4 cc-chunks
            # of each projection can start on those), then x4-7 + h=1 ----
            wdsc = {"q1": wq1_d, "k1": wk1_d, "q2": wq2_d, "k2": wk2_d}

            def w_dma(eng, nm, h):
                for cf in range(2):
                    eng.dma_start(
                        wqm[nm][h][:, cf * 512:(cf + 1) * 512],
                        wdsc[nm][h * 128:(h + 1) * 128,
                                 cf * 512:(cf + 1) * 512])

            def x_dma(eng, i):
                eng.dma_start(xt[i][:], xT_d[i * 128:(i + 1) * 128, :])

            # weights are small (256KB per half) and are needed by every
            # x-chunk matmul: land them first, then stream x in cc order
            w_dma(nc.sync, "q1", 0)
            w_dma(nc.scalar, "k1", 0)
            w_dma(nc.gpsimd, "q2", 0)
            w_dma(nc.sync, "q1", 1)
            w_dma(nc.scalar, "k1", 1)
            w_dma(nc.gpsimd, "k2", 0)
            x_dma(nc.sync, 0)
            x_dma(nc.scalar, 1)
            x_dma(nc.gpsimd, 2)
            x_dma(nc.sync, 3)
            x_dma(nc.scalar, 4)
            x_dma(nc.gpsimd, 5)
            x_dma(nc.sync, 6)
            x_dma(nc.scalar, 7)
            w_dma(nc.gpsimd, "q2", 1)
            w_dma(nc.gpsimd, "k2", 1)
            nc.gpsimd.dma_start(mask_t[:], mask_d)
            nc.scalar.dma_start(ident_t[:], ident_d)
            nc.gpsimd.dma_start(lamn_t[:], lamn_d)
            # wv needed by the first background v chunks (~35us in)
            for h in range(2):
                for cf in range(2):
                    eng = nc.sync if cf == 0 else nc.scalar
                    eng.dma_start(
                        wvm[h][:, cf * 1024:(cf + 1) * 1024],
                        wv_d[h * 128:(h + 1) * 128,
                             cf * 1024:(cf + 1) * 1024])
            # wc + groupnorm consts needed late
            for j in range(NH):
                eng = (nc.sync, nc.scalar)[j % 2]
                eng.dma_start(wct[j][:], wc_d[j * 128:(j + 1) * 128, :])
            nc.gpsimd.dma_start(gg_t[:], gg_d)
            nc.gpsimd.dma_start(gw2_t[:], gw2_d)
            nc.gpsimd.dma_start(gb2_t[:], gb2_d)
            nc.gpsimd.dma_start(icon_t[:], icon_d)

            # qT/kT projections: out [o=128, t=512] = W^T_chunk.T @ xT
            _qk_ps = {}

            def emit_qk(nm, dst, oc, half):
                if half == 0:
                    _qk_ps[(nm, oc)] = (ps_s.tile([128, 1024], F32, tag="s", name="qkA"),
                                        ps_s.tile([128, 1024], F32, tag="s", name="qkB"))
                psA, psB = _qk_ps[(nm, oc)]
                for cc in range(4 * half, 4 * half + 4):
                    for tb in range(T // 512):
                        ph = (psA, psB)[tb // 2]
                        mi = nc.tensor.matmul(
                            ph[:, (tb % 2) * 512:(tb % 2) * 512 + 512],
                            wq_sl(nm, cc, oc),
                            xt[cc][:, tb * 512:(tb + 1) * 512],
                            start=(cc == 0), stop=(cc == 7),
                            skip_group_check=True)
                        if tb > 0:
                            mi.ins.ldweights = False
                if half == 1:
                    for tb2 in range(2):
                        src = (psA, psB)[tb2][:]
                        if nm in ("q1", "k1"):
                            nc.scalar.copy(
                                dst[oc][:, tb2 * 1024:(tb2 + 1) * 1024], src)
                        else:
                            nc.vector.tensor_copy(
                                dst[oc][:, tb2 * 1024:(tb2 + 1) * 1024], src)

            def emit_qk_full(nm, dst, oc):
                emit_qk(nm, dst, oc, 0)
                emit_qk(nm, dst, oc, 1)

            # oc=0 projections up front (heads 0-1 attention needs them)
            for nm, dst in (("q1", q1t), ("k1", k1t), ("q2", q2t), ("k2", k2t)):
                emit_qk_full(nm, dst, 0)

            # v projection chunks are emitted lazily
            v_done = [False] * NT

            def emit_v(tch):
                if v_done[tch]:
                    return
                v_done[tch] = True
                ps = ps_t.tile([128, NH * D], F32, tag="tz")
                for cc in range(8):
                    nc.tensor.matmul(
                        ps[:],
                        xt[cc][:, tch * 128:(tch + 1) * 128],
                        wv_sl(cc),
                        start=(cc == 0), stop=(cc == 7))
                if tch % 2 == 0:
                    nc.vector.tensor_copy(vt[tch][:], ps[:])
                else:
                    nc.scalar.copy(vt[tch][:], ps[:])

            # background PE work drained between attention phases of
            # heads 0-1: v chunks (just-in-time via the phase2 safety),
            # then the oc=1 projections (needed only by pair (2,3))
            bg = [(lambda t=t: emit_v(t)) for t in range(NT)]
            for nm, dst in (("q1", q1t), ("k1", k1t),
                            ("q2", q2t), ("k2", k2t)):
                bg.append(lambda nm=nm, dst=dst: emit_qk_full(nm, dst, 1))

            def bg_drain(n):
                for _ in range(min(n, len(bg))):
                    bg.pop(0)()

            # ================= attention per head =================
            head_ctx = {}

            def phase1_qt(j, qb, qq, AB):
                oc, po = divmod(j * HS, 128)
                qt = qb * 4 + qq
                nk = qt + 1
                nkb2 = (nk + 7) // 8   # 1024-wide S psum tiles
                e1 = wp.tile([128, T], BF16, tag="e1", name="e1", bufs=4)
                e2 = wp.tile([128, T], BF16, tag="e2", name="e2", bufs=3)
                dd = wp.tile([128, 2], F32, tag="dd", name="dd", bufs=4)
                d1c = wp.tile([128, 2], F32, tag="d1c", name="d1c", bufs=4)
                d2c = wp.tile([128, 2], F32, tag="d2c", name="d2c", bufs=4)
                for mi, (qsrc, ksrc, erow, dcol) in enumerate(
                        ((q1t, k1t, e1, d1c), (q2t, k2t, e2, d2c))):
                    for kb in range(nkb2):
                        w = min(1024, nk * 128 - kb * 1024)
                        ps = ps_s.tile([128, 1024], F32, tag="s",
                                       name="ps")
                        off = qt * 128 - kb * 1024  # diag block col
                        for hf in range(2):
                            wh = min(512, w - hf * 512)
                            if wh <= 0:
                                break
                            diag_here = (kb == nkb2 - 1 and
                                         hf * 512 <= off < hf * 512 + wh)
                            mm = nc.tensor.matmul(
                                ps[:, hf * 512:hf * 512 + wh],
                                qsrc[oc][po:po + HS,
                                         qt * 128:(qt + 1) * 128],
                                ksrc[oc][po:po + HS,
                                         kb * 1024 + hf * 512:
                                         kb * 1024 + hf * 512 + wh],
                                start=True, stop=not diag_here,
                                skip_group_check=diag_here)
                            if kb + hf > 0:
                                mm.ins.ldweights = False
                        if kb == nkb2 - 1:
                            # mask diagonal 128-block on PE
                            nc.tensor.matmul(
                                ps[:, off:off + 128],
                                ident_t[:], mask_t[:],
                                start=False, stop=True,
                                skip_group_check=True)
                        # accum straight into dd when a single psum covers
                        # the row (saves two DVE copies per qt)
                        acc = (dd[:, mi:mi + 1] if nkb2 == 1
                               else dcol[:, kb:kb + 1])
                        nc.scalar.activation(
                            erow[:, kb * 1024:kb * 1024 + w],
                            ps[:, :w], AF.Exp, scale=SCALE,
                            accum_out=acc)
                # denominators -> rr = [1/D1, 1/D2]
                rr = wp.tile([128, 2], F32, tag="rr", name="rr", bufs=4)
                if nkb2 > 1:
                    nc.vector.tensor_reduce(
                        dd[:, 0:1], d1c[:, 0:nkb2],
                        axis=mybir.AxisListType.X, op=ALU.add)
                    nc.vector.tensor_reduce(
                        dd[:, 1:2], d2c[:, 0:nkb2],
                        axis=mybir.AxisListType.X, op=ALU.add)
                nc.vector.reciprocal(rr[:], dd[:, 0:2])
                # sc2 = -lam / D2 (per-partition scalar)
                sc2 = wp.tile([128, 1], F32, tag="sc2", name="sc2", bufs=4)
                nc.vector.tensor_tensor(sc2[:], rr[:, 1:2],
                                        lamn_t[:, j:j + 1], ALU.mult)
                # att = e1*r1 + e2*sc2 (fully normalized diff-attention
                # row): tensor_scalar (4x) + scalar_tensor_tensor
                etmp = wp.tile([128, T], BF16, tag="etmp", name="etmp",
                               bufs=1)
                nc.vector.tensor_scalar(
                    etmp[:, :nk * 128], e2[:, :nk * 128], sc2[:, 0:1],
                    None, op0=ALU.mult)
                nc.vector.scalar_tensor_tensor(
                    e1[:, :nk * 128], e1[:, :nk * 128], rr[:, 0:1],
                    etmp[:, :nk * 128], op0=ALU.mult, op1=ALU.add)
                # transpose this attention row on the DMA xbar:
                # [q=128, nk*128] -> nk transposed blocks [k=128, 128]
                nc.sync.dma_start_transpose(
                    AB[:, 0:nk, qq, :], e1[:, :nk * 128])

            def phase1_pair(jA, jB, qb):
                # qt-interleaved emission across the head pair: the PE always
                # has the other head's independent S-chunk while ACT/DVE
                # drain this one's exp/combine chain
                ABs = {}
                for j in (jA, jB):
                    # AB layout [k=128, kc, qq, q-col]: z-matmul rhs slices
                    # contiguous (strided moving operands are silently wrong
                    # on HW; strided DMA-transpose OUT is fine)
                    ABs[j] = wp.tile([128, NT, 4, 128], BF16, tag="AB",
                                     name="AB", bufs=2)
                for qq in range(4):
                    for j in (jA, jB):
                        phase1_qt(j, qb, qq, ABs[j])
                return ABs

            def phase2(j, qb, AB):
                ytr, s1p, s2p = head_ctx[j]
                nkc = qb * 4 + 4
                # yT[d, qblk] = sum_kc v_kc.T @ attT_kc   (N=512)
                py = ps_t.tile([128, 512], F32, tag="tz", name="py")
                for kc in range(nkc):
                    emit_v(kc)
                    qq0 = max(0, kc - qb * 4)
                    zw = qq0 * 128
                    nc.tensor.matmul(
                        py[:, zw:],
                        vt[kc][:, j * 128:(j + 1) * 128],
                        AB[:, kc, qq0:4, :].rearrange("p q c -> p (q c)"),
                        start=(kc == 0), stop=(kc == nkc - 1),
                        skip_group_check=True)
                # copy to ytr with fused stats accumulation on DVE (no
                # 183ns ACT accumulator-read tax; ACT is the busier engine)
                nc.vector.tensor_scalar(
                    ytr[:, qb * 512:(qb + 1) * 512], py[:], 1.0, 0.0,
                    op0=ALU.mult, op1=ALU.add,
                    accum_out=s1p[:, qb:qb + 1])
                ysq = wp.tile([128, 512], BF16, tag="ysq", name="ysq",
                              bufs=1)
                ysrc = ytr[:, qb * 512:(qb + 1) * 512]
                nc.vector.scalar_tensor_tensor(
                    ysq[:], ysrc, 1.0, ysrc,
                    op0=ALU.mult, op1=ALU.mult,
                    accum_out=s2p[:, qb:qb + 1])

            def gn_final(j):
                ytr, s1p, s2p = head_ctx[j]
                s12 = wp.tile([128, 2], F32, tag="s12", name="s12")
                nc.vector.tensor_reduce(s12[:, 0:1], s1p[:, 0:NT // 4],
                                        axis=mybir.AxisListType.X, op=ALU.add)
                nc.vector.tensor_reduce(s12[:, 1:2], s2p[:, 0:NT // 4],
                                        axis=mybir.AxisListType.X, op=ALU.add)
                pg = ps_t.tile([128, 2], F32, tag="tz", name="pg")
                nc.tensor.matmul(pg[:], gg_t[:], s12[:], start=True, stop=True)
                # mneg = -mean; nvar = mean^2 - E[y^2] = -var
                mneg = wp.tile([128, 1], F32, tag="mneg", name="mneg")
                nc.scalar.mul(mneg[:], pg[:, 0:1], -1.0 / (T * 4))
                msq = wp.tile([128, 1], F32, tag="msq")
                nc.scalar.mul(msq[:], pg[:, 1:2], 1.0 / (T * 4))
                nvar = wp.tile([128, 1], F32, tag="nvar")
                nc.vector.scalar_tensor_tensor(
                    nvar[:], mneg[:], mneg[:, 0:1], msq[:],
                    op0=ALU.mult, op1=ALU.subtract)
                vpe = wp.tile([128, 1], F32, tag="vpe")
                nc.vector.tensor_scalar(vpe[:], nvar[:], -1.0, EPS,
                                        op0=ALU.mult, op1=ALU.add)  # var+eps
                # rsqrt(var+eps) on DVE only: quake seed + Newton iters
                rstd = wp.tile([128, 1], F32, tag="rstd")
                yi = wp.tile([128, 1], F32, tag="yi")
                nc.vector.tensor_tensor(yi.bitcast(mybir.dt.uint32)[:],
                                        vpe.bitcast(mybir.dt.uint32)[:],
                                        icon_t[:, 0:1],
                                        ALU.logical_shift_right)
                nc.vector.tensor_tensor(yi.bitcast(mybir.dt.uint32)[:],
                                        icon_t[:, 1:2],
                                        yi.bitcast(mybir.dt.uint32)[:],
                                        ALU.subtract)
                vneg = wp.tile([128, 1], F32, tag="vneg")
                nc.vector.tensor_scalar_mul(vneg[:], vpe[:], -0.5)
                ytmp = wp.tile([128, 1], F32, tag="ytmp")
                for _ in range(2):
                    nc.vector.tensor_tensor(ytmp[:], yi[:], yi[:], ALU.mult)
                    nc.vector.scalar_tensor_tensor(
                        ytmp[:], ytmp[:], vneg[:, 0:1], c15_t[:],
                        op0=ALU.mult, op1=ALU.add)  # 1.5 - 0.5 v y^2
                    nc.vector.tensor_tensor(yi[:], yi[:], ytmp[:], ALU.mult)
                nc.vector.tensor_copy(rstd[:], yi[:])
                aff_a = wp.tile([128, 1], F32, tag="aff_a")
                nc.vector.tensor_tensor(aff_a[:], rstd[:], gw2_t[:], ALU.mult)
                aff_b = wp.tile([128, 1], F32, tag="aff_b")
                nc.vector.scalar_tensor_tensor(
                    aff_b[:], mneg[:], aff_a[:, 0:1], gb2_t[:],
                    op0=ALU.mult, op1=ALU.add)  # gb2 - mean*aff_a
                # affine on DVE (4x): yt = ytr*aff_a + aff_b
                nc.vector.tensor_scalar(yt[j][:], ytr[:], aff_a[:, 0:1],
                                        aff_b[:, 0:1],
                                        op0=ALU.mult, op1=ALU.add)

            def new_head(j):
                head_ctx[j] = (
                    wp.tile([128, T], BF16, tag="ytr", name="ytr", bufs=3),
                    wp.tile([128, 4], F32, tag="s1p", name="s1p"),
                    wp.tile([128, 4], F32, tag="s2p", name="s2p"))

            # qb order (1,2,3,0): the serial end-of-pair chain (last exp ->
            # combine -> transpose -> z -> stats -> gn) runs on the smallest
            # q-block, shrinking the pair-boundary latency
            QBS = (1, 2, 3, 0)

            # ---- pair (0,1): bg (v + oc1 projections) fills the PE ----
            for j in (0, 1):
                new_head(j)
            for qb in QBS:
                ABs = phase1_pair(0, 1, qb)
                bg_drain(5)
                phase2(0, qb, ABs[0])
                if qb == 0:
                    bg_drain(len(bg))
                    gn_final(0)
                phase2(1, qb, ABs[1])
                if qb == 0:
                    gn_final(1)

            # pair boundary: release x/weight tiles, open the partial pool
            lp_cm.__exit__(None, None, None)
            dp_cm = tc.tile_pool(name="drain", bufs=1)
            dp = dp_cm.__enter__()
            p01 = [dp.tile([128, T], BF16, tag=f"p01_{ocb}",
                           name=f"p01_{ocb}") for ocb in range(8)]

            # out-proj j=0,1 partials: fill PE during the ACT-bound
            # (2,3) pair. Two ocbs per qb slot.
            fill_q = list(range(8))

            def emit_fill(ocb):
                for tb in range(T // 512):
                    pt = ps_t.tile([128, 512], F32, tag="tz", name="fl")
                    for j in (0, 1):
                        nc.tensor.matmul(
                            pt[:],
                            wct[j][:, ocb * 128:(ocb + 1) * 128],
                            yt[j][:, tb * 512:(tb + 1) * 512],
                            start=(j == 0), stop=(j == 1),
                            skip_group_check=True)
                    nc.vector.tensor_copy(
                        p01[ocb][:, tb * 512:(tb + 1) * 512], pt[:])

            # ---- pair (2,3): fills + attention ----
            for j in (2, 3):
                new_head(j)
            for qb in QBS:
                ABs = phase1_pair(2, 3, qb)
                for _ in range(2):
                    if fill_q:
                        emit_fill(fill_q.pop(0))
                phase2(2, qb, ABs[2])
                if qb == 0:
                    while fill_q:
                        emit_fill(fill_q.pop(0))
                    gn_final(2)
                phase2(3, qb, ABs[3])
                if qb == 0:
                    gn_final(3)

            # ================= output projection =================
            # per ocb: ident-add of the j01 partial, then j=2, then j=3.
            # gn(3) is emitted just before ocb0 so its ACT->DVE chain hides
            # behind the gn-independent ident-adds + j=2 matmuls.
            def s_halves():
                psA = ps_s.tile([128, 1024], F32, tag="s", name="psA")
                psB = ps_s.tile([128, 1024], F32, tag="s", name="psB")
                return (psA, psB)

            def op_add(ocb, halves):
                for tb in range(T // 512):
                    ph = halves[tb // 2]
                    mi = nc.tensor.matmul(
                        ph[:, (tb % 2) * 512:(tb % 2) * 512 + 512],
                        ident_t[:],
                        p01[ocb][:, tb * 512:(tb + 1) * 512],
                        start=True, stop=False,
                        skip_group_check=True)
                    if tb > 0:
                        mi.ins.ldweights = False

            def op_mms23(ocb, halves):
                for j in (2, 3):
                    for tb in range(T // 512):
                        ph = halves[tb // 2]
                        mi = nc.tensor.matmul(
                            ph[:, (tb % 2) * 512:(tb % 2) * 512 + 512],
                            wct[j][:, ocb * 128:(ocb + 1) * 128],
                            yt[j][:, tb * 512:(tb + 1) * 512],
                            start=False, stop=(j == 3),
                            skip_group_check=True)
                        if tb > 0:
                            mi.ins.ldweights = False

            _fin_rr = [0]

            def op_fin(ocb, halves):
                for tb2 in range(2):
                    ob = dp.tile([128, 1024], BF16, tag="ob", bufs=4,
                                 name="ob")
                    r = _fin_rr[0] = (_fin_rr[0] + 1) % 2
                    if r == 0:
                        nc.vector.tensor_copy(ob[:], halves[tb2][:])
                    else:
                        nc.scalar.copy(ob[:], halves[tb2][:])
                    eng = (nc.sync, nc.gpsimd, nc.scalar)[(2 * ocb + tb2) % 3]
                    eng.dma_start(
                        outT_d[ocb * 128:(ocb + 1) * 128,
                               tb2 * 1024:(tb2 + 1) * 1024], ob[:])

            for ocb in range(8):
                halves = s_halves()
                op_add(ocb, halves)
                op_mms23(ocb, halves)
                op_fin(ocb, halves)
            dp_cm.__exit__(None, None, None)
            wp_cm.__exit__(None, None, None)

    nc.compile()
    return nc


def _prep_inputs(inputs):
    bf = ml_dtypes.bfloat16
    x = np.asarray(inputs["x"], np.float32)
    Wq1 = np.asarray(inputs["Wq1"], np.float32)
    Wq2 = np.asarray(inputs["Wq2"], np.float32)
    Wk1 = np.asarray(inputs["Wk1"], np.float32)
    Wk2 = np.asarray(inputs["Wk2"], np.float32)
    Wv = np.asarray(inputs["Wv"], np.float32)
    Wc = np.asarray(inputs["Wc"], np.float32)
    gn_w = np.asarray(inputs["gn_w"], np.float32)
    gn_b = np.asarray(inputs["gn_b"], np.float32)
    gamma = np.asarray(inputs["gamma"], np.float32)

    def sig(v):
        return 1.0 / (1.0 + np.exp(-v))

    lam = (sig(np.asarray(inputs["lq1"], np.float32).reshape(H)
               * np.asarray(inputs["lk1"], np.float32).reshape(H))
           - sig(np.asarray(inputs["lq2"], np.float32).reshape(H)
                 * np.asarray(inputs["lk2"], np.float32).reshape(H))
           + LAMBDA_INIT)

    mask = np.where(np.arange(128)[None, :] <= np.arange(128)[:, None],
                    0.0, NEG).astype(bf)
    ident = np.eye(128, dtype=np.float32).astype(bf)
    gg = (np.arange(128)[:, None] // 4 == np.arange(128)[None, :] // 4
          ).astype(np.float32)
    c1 = 1.0 - LAMBDA_INIT
    gw2 = (gn_w * gamma * c1).astype(np.float32).reshape(128, 1)
    gb2 = (gn_b * gamma * c1).astype(np.float32).reshape(128, 1)

    icon = np.zeros((128, 2), np.uint32)
    icon[:, 0] = 1
    icon[:, 1] = 0x5f375a00
    xTb = [np.ascontiguousarray(x[b].T).astype(bf) for b in range(B)]
    in_maps = []
    for core in range(N_CORES):
        b, hg = divmod(core, N_CORES // B)
        qs = hg * NH * HS          # 256-wide q/k slice
        vs = hg * NH * D           # 512-wide v / y2 slice
        lamn = np.repeat(-lam[hg * NH:(hg + 1) * NH].reshape(1, NH),
                         128, axis=0).astype(np.float32)
        def mtiles(wt, cols):
            # [1024, cols] -> merged 2-tile layout [256, 4*cols]
            return np.ascontiguousarray(
                wt.reshape(2, 4, 128, cols).transpose(0, 2, 1, 3)
                .reshape(256, 4 * cols)).astype(bf)

        in_maps.append({
            "xT": xTb[b],
            "wq1T": mtiles(Wq1[qs:qs + NH * HS, :].T, NH * HS),
            "wq2T": mtiles(Wq2[qs:qs + NH * HS, :].T, NH * HS),
            "wk1T": mtiles(Wk1[qs:qs + NH * HS, :].T, NH * HS),
            "wk2T": mtiles(Wk2[qs:qs + NH * HS, :].T, NH * HS),
            "wvT": mtiles(Wv[vs:vs + NH * D, :].T, NH * D),
            "wcT": np.ascontiguousarray(Wc[:, vs:vs + NH * D].T).astype(bf),
            "mask128": mask,
            "ident": ident,
            "gg": gg,
            "gw2": gw2,
            "gb2": gb2,
            "lamn": lamn,
            "icon": icon,
        })
    return in_maps


def kernel(**inputs):
    if "nc" not in _cache:
        _cache["nc"] = _build()
    nc = _cache["nc"]
    in_maps = _prep_inputs(inputs)
    res = bass_utils.run_bass_kernel_spmd(
        nc, in_maps, core_ids=list(range(N_CORES)),
        **_cache.get("run_kwargs", {}))
    _cache["last_result"] = res
    out = np.zeros((B, T, C), np.float32)
    for core in range(N_CORES):
        b = core // (N_CORES // B)
        out[b] += res.results[core]["outT"].T.astype(np.float32)
    return out
